# revision 73
# baseline (speedup 1.0000x reference)
"""Trainium2 Bass kernel for single-head self-attention (B=2, S=4096, D=1024).

reference:
    q = x @ Wq; k = x @ Wk; v = x @ Wv          # [B,S,D]
    energy = einsum('bid,bjd->bij', q, k) * 8.0  # SCALE = sqrt(64)
    attn = softmax(energy, axis=-1)
    out = einsum('bij,bjd->bid', attn, v) @ Wo

Two SPMD launches over 8 cores (= 2 batches x 4 query-blocks of 1024):
  phase 1: each core computes the Q/K/V projections for its own 1024
           rows only (1/8 of the total work, no redundancy); the host
           gathers K/V shards into full per-batch tensors.
  phase 2: each core runs attention + output projection for its block
           against the full K/V of its batch.

Precision: logits have std ~256 (SCALE multiplies), so softmax is
nearly an argmax -- the x->Q, x->K, Q@K^T path needs ~2^-12 relative
accuracy.  fp16 (e5m10) gives 2^-11.8 rounding at the same 1
cycle/row PE rate as bf16, and Q/K values are ~N(0,1) so the e5
range is ample; those matmuls run as a single fp16 pass with fp32
PSUM accumulation (end-to-end rel err ~8e-3, dominated by the
softmax's sensitivity to logit rounding; bf16 single-pass would be
~5e-2, and the old bf16 hi/lo 3-pass scheme costs 3x the cycles).
The V projection tolerates ~0.5% error, so it runs entirely as fp8
e4m3 DoubleRow matmuls (0.5 cycles/row, two k-tiles contracted per
instruction = 4x bf16 MAC rate in the cost model): hi*hi plus the
two hi*lo cross terms, with the lo planes pre-scaled by 32 to clear
the e4m3 denormal floor and the 1/32 folded back during the
PSUM->SBUF merge (scalar-engine Copy-with-scale + DVE add).

Phase 2 is software-pipelined: each iteration emits softmax(i)'s
Act/DVE chain, then E(i+1)'s matmuls (which keep the PE busy through
the softmax latency), then P^T transposes and P@V for i.  K, V and
the full output projection Wo path are scheduled so V stays resident
in SBUF all phase (the fp16/fp8 shrink of K freed the room), and the
output projection runs as column strips woven into the E sweeps'
PSUM-drain slots, with the last strip packed 4-chunks-per-bank into
the by-then-idle PV banks.

Layout: feature-major ("transposed") activations throughout; the host
pre-transposes x and post-transposes the output. DMA instruction count
is kept low (batched loads/stores) -- each HWDGE descriptor-generation
costs ~0.65us of serialized queue-prep time on its issuing engine, and
all transfers serialize on the DMA-engine pool at ~360 GB/s, so issue
order is chosen to match consumption order (prologue loads split
across the SP and Activation HWDGE queues).
"""

import numpy as np

B, S, D = 2, 4096, 1024
BLK = 1024          # queries per core
SCALE = 8.0         # HEAD_DIM ** 0.5 = sqrt(64)
NK = D // 128       # 8 k-tiles over the feature dim
NT = S // 128       # 32 j-tiles over keys
NI = BLK // 128     # 8 i-tiles over this core's queries
NJB = S // 512      # 8 key blocks of 512
F16 = np.float16
import ml_dtypes
E4M3 = ml_dtypes.float8_e4m3
LO_SCALE = np.float32(32.0)   # pre-scale on e4m3 lo planes (kept out of denormals)


def _split_e4m3(a):
    """fp32 -> (hi, lo) e4m3 planes with lo pre-scaled by LO_SCALE."""
    hi = a.astype(E4M3)
    lo = ((a - hi.astype(np.float32)) * LO_SCALE).astype(E4M3)
    return hi, lo

# phase-2 tuning knobs (swept via TimelineSim); PSUM is 8 banks total:
# EPS + TP + 2 (op0/op1) + YPS <= 8
EPS_BUFS = 3        # PSUM banks for E accumulation
TP_BUFS = 2         # PSUM banks for PE transposes
YPS_BUFS = 1        # PSUM banks for Y (out-projection) accumulation
DUAL_Q = True       # split prologue DMA issue across SP + Act HWDGE queues
FIRST_BAND_512 = False  # 512-col leading K bands instead of 1024
WO_RESIDENT = False  # keep all of Wo in SBUF instead of streaming per strip
QV23 = True         # preload qv2/qv3 on the Act queue
STRIP_MID = True    # Y strips ride as E-sweep fillers (see `fillers`)

_cache = {}


def _build_phase1():
    """Q/K/V projections for this core's 1024 rows."""
    import concourse.mybir as mybir
    from concourse import bacc
    from concourse.tile import TileContext

    FP32 = mybir.dt.float32
    DF16 = mybir.dt.float16
    DF8 = mybir.dt.float8e4
    DR = mybir.MatmulPerfMode.DoubleRow
    ADD = mybir.AluOpType.add
    Copy = mybir.ActivationFunctionType.Copy

    nc = bacc.Bacc("TRN2", target_bir_lowering=False, debug=False, num_devices=8)

    xt = nc.dram_tensor("xt", [D, BLK], DF16, kind="ExternalInput")   # rows.T
    # e4m3 2-level x/Wv for the V projection; lo planes pre-scaled by 32
    x8 = nc.dram_tensor("x8", [128, NK, 2, BLK], DF8, kind="ExternalInput")
    wv8 = nc.dram_tensor("wv8", [128, NK, 2, D], DF8, kind="ExternalInput")
    wq = nc.dram_tensor("wq", [D, D], DF16, kind="ExternalInput")
    wk = nc.dram_tensor("wk", [D, D], DF16, kind="ExternalInput")
    qt = nc.dram_tensor("qt", [D, BLK], DF16, kind="ExternalOutput")
    kt = nc.dram_tensor("kt", [D, BLK], DF16, kind="ExternalOutput")
    vo = nc.dram_tensor("vo", [NI, 128, D], DF16, kind="ExternalOutput")

    with TileContext(nc) as tc:
      with (
          tc.tile_pool(name="xqk", bufs=1) as xqkp,
          tc.tile_pool(name="wkqp", bufs=1) as wkqp,
          tc.tile_pool(name="wvp", bufs=1) as wvp,
          tc.tile_pool(name="vps", bufs=2, space="PSUM") as vps,
          tc.tile_pool(name="vsb", bufs=3) as vsbp,
      ):
        if True:
            # load order = consumption order; first K-weight sliver and x
            # halves first so the K sweep starts as early as possible
            xt_r = xt[:, :].rearrange("(n p) s -> p n s", p=128)
            xs = xqkp.tile([128, NK, BLK], DF16, name="xs", tag="xs")
            w_sb = {}
            wk_sb = wkqp.tile([128, NK, D], DF16, name="kwt", tag="kwt")
            wk_r = wk[:, :].rearrange("(n p) d -> p n d", p=128)
            nc.scalar.dma_start(wk_sb[:, :, 0:128], wk_r[:, :, 0:128])
            nc.sync.dma_start(xs[:, :, 0:128], xt_r[:, :, 0:128])
            nc.sync.dma_start(xs[:, :, 128:256], xt_r[:, :, 128:256])
            nc.scalar.dma_start(wk_sb[:, :, 128:512], wk_r[:, :, 128:512])
            nc.sync.dma_start(xs[:, :, 256:512], xt_r[:, :, 256:512])
            nc.sync.dma_start(xs[:, :, 512:BLK], xt_r[:, :, 512:BLK])
            nc.scalar.dma_start(wk_sb[:, :, 512:D], wk_r[:, :, 512:D])
            w_sb["k"] = wk_sb
            x8_sb = wvp.tile([128, NK, 2, BLK], DF8, name="x8t", tag="x8t")
            nc.scalar.dma_start(x8_sb, x8[:, :, :, :])
            wv8_sb = wvp.tile([128, NK, 2, D], DF8, name="wv8t", tag="wv8t")
            nc.sync.dma_start(wv8_sb, wv8[:, :, :, :])
            wq_sb = wkqp.tile([128, NK, D], DF16, name="qwt", tag="qwt")
            wq_r = wq[:, :].rearrange("(n p) d -> p n d", p=128)
            nc.scalar.dma_start(wq_sb[:, :, 0:512], wq_r[:, :, 0:512])
            nc.sync.dma_start(wq_sb[:, :, 512:D], wq_r[:, :, 512:D])
            w_sb["q"] = wq_sb
        # ------------- Kt, then V, then Qt -------------
        if True:
            def v_section():
                # V = x @ Wv in 2-level e4m3: hi*hi into one PSUM bank, the
                # two cross terms (carrying the x32 lo pre-scale) into a
                # second, merged by one (cross/32 + main) DVE op.  All four
                # products run as fp8 DoubleRow matmuls (0.5 cycles/row,
                # k-tile pairs), 0.75x the fp16 cost.
                NH = NK // 2
                for j in range(NI):
                    vt = vsbp.tile([128, D], DF16, name=f"vt{j}", tag="vt")
                    jsl = slice(j * 128, (j + 1) * 128)
                    for db in range(2):
                        dsl = slice(db * 512, (db + 1) * 512)
                        psm = vps.tile([128, 512], FP32, name=f"vpm{j}_{db}", tag="vpm")
                        psc = vps.tile([128, 512], FP32, name=f"vpc{j}_{db}", tag="vpc")
                        for kp in range(NH):
                            k2 = slice(2 * kp, 2 * kp + 2)
                            nc.tensor.matmul(
                                psm, lhsT=x8_sb[:, k2, 0, jsl],
                                rhs=wv8_sb[:, k2, 0, dsl], perf_mode=DR,
                                start=(kp == 0), stop=(kp == NH - 1))
                        for kp in range(NH):
                            k2 = slice(2 * kp, 2 * kp + 2)
                            nc.tensor.matmul(
                                psc, lhsT=x8_sb[:, k2, 0, jsl],
                                rhs=wv8_sb[:, k2, 1, dsl], perf_mode=DR,
                                start=(kp == 0), stop=False)
                            nc.tensor.matmul(
                                psc, lhsT=x8_sb[:, k2, 1, jsl],
                                rhs=wv8_sb[:, k2, 0, dsl], perf_mode=DR,
                                start=False, stop=(kp == NH - 1))
                        # cross/32 on the (idle) scalar engine, then one DVE add
                        # (walrus rejects a dual-PSUM scalar_tensor_tensor)
                        vc = vsbp.tile([128, 512], DF16, name=f"vc{j}_{db}", tag="vc")
                        nc.scalar.activation(vc, psc, Copy, scale=float(1.0 / LO_SCALE))
                        nc.vector.tensor_tensor(vt[:, dsl], psm, vc, op=ADD)
                    nc.sync.dma_start(vo[j], vt)

            # K's leading column-chunks are narrow so its first PSUM group
            # only needs 128 columns of x to have landed
            CHUNKS = {"k": ((0, 128), (128, 256), (256, 384), (384, 512), (512, 1024)),
                      "q": ((0, 512), (512, 1024))}
            for (outd, pfx) in ((kt, "k"), (qt, "q")):
                with (
                    tc.tile_pool(name=f"{pfx}ps", bufs=4, space="PSUM") as psp,
                    tc.tile_pool(name=f"{pfx}st", bufs=1) as stp,
                ):
                    w_t = w_sb[pfx]
                    sh_t = []
                    for m in range(NK):
                        sh_t.append(stp.tile([128, BLK], DF16, name=f"{pfx}sh{m}", tag=f"sh{m}"))
                    chunks = CHUNKS[pfx]
                    lc = len(chunks) - 1
                    if pfx == "k":
                        # ordered by operand arrival: narrow x chunks against
                        # the first weight columns first, high-m columns once
                        # the rest of the weight lands, then the wide chunk
                        order = ([(ci, m) for ci in range(4) for m in range(4)]
                                 + [(ci, m) for ci in range(4) for m in range(4, NK)]
                                 + [(lc, m) for m in range(NK)])
                    else:
                        order = [(ci, m) for ci in range(len(chunks)) for m in range(NK)]
                    for (ci, m) in order:
                        c0, c1 = chunks[ci]
                        nsl = slice(c0, c1)
                        msl = slice(m * 128, (m + 1) * 128)
                        ps = psp.tile([128, 512], FP32, name=f"{pfx}ps{ci}_{m}", tag="ps")
                        for k in range(NK):
                            nc.tensor.matmul(ps[:, 0:c1 - c0], lhsT=w_t[:, k, msl],
                                             rhs=xs[:, k, nsl],
                                             start=(k == 0), stop=(k == NK - 1))
                        nc.vector.tensor_copy(sh_t[m][:, nsl], ps[:, 0:c1 - c0])
                        if ci == lc:
                            # store as soon as the last chunk of m is done
                            nc.sync.dma_start(outd[msl, :], sh_t[m])
                if pfx == "k":
                    v_section()
    nc.compile()
    return nc


def _build_phase2():
    """Attention + output projection for this core's 1024 queries."""
    import concourse.mybir as mybir
    from concourse import bacc
    from concourse.tile import TileContext
    from concourse.masks import make_identity

    FP32 = mybir.dt.float32
    DF16 = mybir.dt.float16
    Exp = mybir.ActivationFunctionType.Exp
    AX = mybir.AxisListType.X

    nc = bacc.Bacc("TRN2", target_bir_lowering=False, debug=False, num_devices=8)

    kth = nc.dram_tensor("kth", [D, S], DF16, kind="ExternalInput")
    # per-i-tile partition-major Q: [i, p, n, f] = qt[n*128+p, i*128+f]
    qt1 = nc.dram_tensor("qt1", [NI, 128, NK, 128], DF16, kind="ExternalInput")
    vin = nc.dram_tensor("vin", [NT, 128, D], DF16, kind="ExternalInput")
    # per-m-chunk partition-major Wo: [m, p, n, f] = Wo[n*128+p, m*128+f]
    wo = nc.dram_tensor("wo", [NK, 128, NK, 128], DF16, kind="ExternalInput")
    yt = nc.dram_tensor("yt", [D, BLK], DF16, kind="ExternalOutput")

    from contextlib import ExitStack
    with TileContext(nc) as tc:
        with ExitStack() as stack:
            constp = stack.enter_context(tc.tile_pool(name="const", bufs=1))
            ident = constp.tile([128, 128], DF16)
            make_identity(nc, ident)

            ktp = stack.enter_context(tc.tile_pool(name="ktp", bufs=1))
            qtp = stack.enter_context(tc.tile_pool(name="qtp", bufs=3))
            if True:
                # first two i-tiles' Q loads before the big K transfers
                qv0 = qtp.tile([128, NK, 128], DF16, name="qv0", tag="qv")
                nc.sync.dma_start(qv0, qt1[0])
                qv1 = qtp.tile([128, NK, 128], DF16, name="qv1", tag="qv")
                nc.sync.dma_start(qv1, qt1[1])
                # column-banded K loads, issued on BOTH HWDGE queues (SP +
                # Activation) to halve the ~0.65us/DMA descriptor-gen
                # serialization in the prologue; V (resident all phase) and
                # Wo (resident) follow
                kth_sb = []
                for m in range(NK):
                    kth_sb.append(ktp.tile([128, S], DF16, name=f"kth{m}", tag=f"kth{m}"))
                vrp = stack.enter_context(tc.tile_pool(name="vrp", bufs=1))
                vres = vrp.tile([128, NT, D], DF16, name="vres", tag="vres")
                vin_r = vin[:, :, :].rearrange("t p d -> p t d")
                if FIRST_BAND_512:
                    bands = ((0, 512), (512, 1024), (1024, 2048),
                             (2048, 3072), (3072, 4096))
                else:
                    bands = ((0, 1024), (1024, 2048),
                             (2048, 3072), (3072, 4096))
                for bi, (c0, c1) in enumerate(bands):
                    csl = slice(c0, c1)
                    for m in range(NK):
                        eng = nc.scalar if (DUAL_Q and bi < 2 and m >= NK // 2) else nc.sync
                        eng.dma_start(kth_sb[m][:, csl], kth[m * 128:(m + 1) * 128, csl])
                for cb in range(4):
                    tsl = slice(cb * (NT // 4), (cb + 1) * (NT // 4))
                    nc.sync.dma_start(vres[:, tsl], vin_r[:, tsl])
                # everything below is needed only from the middle of the
                # phase on -- keep it BEHIND the V quarters in the DMA FIFO
                qv23 = []
                if QV23:
                    for i in (2, 3):
                        qvn = qtp.tile([128, NK, 128], DF16, name=f"qv{i}", tag="qv")
                        (nc.scalar if DUAL_Q else nc.sync).dma_start(qvn, qt1[i])
                        qv23.append(qvn)
                wo_sb = None
                if WO_RESIDENT:
                    wop = stack.enter_context(tc.tile_pool(name="wop", bufs=1))
                    wo_sb = wop.tile([128, NK, NK, 128], DF16, name="wos", tag="wos")
                    eng = nc.scalar if DUAL_Q else nc.sync
                    eng.dma_start(wo_sb[:, 0:NK // 2], wo[0:NK // 2].rearrange("m p n f -> p m n f"))
                    eng.dma_start(wo_sb[:, NK // 2:NK], wo[NK // 2:NK].rearrange("m p n f -> p m n f"))
                else:
                    wop = stack.enter_context(tc.tile_pool(name="wop", bufs=2))

                otp = stack.enter_context(tc.tile_pool(name="otp", bufs=1))
                if True:
                    ot_sb = []
                    for t in range(NK):
                        ot_sb.append(otp.tile([128, BLK], DF16, name=f"ot{t}", tag=f"ot{t}"))
                    epsp = stack.enter_context(tc.tile_pool(name="eps", bufs=EPS_BUFS, space="PSUM"))
                    tpsp = stack.enter_context(tc.tile_pool(name="tps", bufs=TP_BUFS, space="PSUM"))
                    opsp = stack.enter_context(tc.tile_pool(name="ops", bufs=1, space="PSUM"))
                    smp = stack.enter_context(tc.tile_pool(name="smp", bufs=2))
                    esp = stack.enter_context(tc.tile_pool(name="esp", bufs=2))
                    pp = stack.enter_context(tc.tile_pool(name="pp", bufs=1))
                    ptp = stack.enter_context(tc.tile_pool(name="ptp", bufs=1))
                    obp = stack.enter_context(tc.tile_pool(name="obp", bufs=1))
                    ypsp = stack.enter_context(tc.tile_pool(name="yps", bufs=YPS_BUFS, space="PSUM"))
                    ystp = stack.enter_context(tc.tile_pool(name="yst", bufs=2))
                    if True:
                        qvt = {0: qv0, 1: qv1}
                        for j, t in enumerate(qv23):
                            qvt[2 + j] = t

                        def get_qv(i):
                            if i not in qvt:
                                qvt[i] = qtp.tile([128, NK, 128], DF16, name=f"qv{i}", tag="qv")
                                nc.sync.dma_start(qvt[i], qt1[i])
                            return qvt[i]

                        def e_sweep(i, filler=()):
                            """E = Q_i @ K^T for one i-tile (raw logits + per-block
                            maxes).  One `filler` callable runs after each j-block's
                            matmuls -- deferred PE work (Y chunks) slots in while
                            the block's PSUM drains, hiding its bank turnaround."""
                            qv = get_qv(i)
                            if i + 1 < NI:
                                get_qv(i + 1)  # prefetch next Q tile
                            mx8 = smp.tile([128, NJB], FP32, name=f"mx8_{i}", tag="mx8")
                            e_sb = esp.tile([128, S], FP32, name=f"e{i}", tag="e")
                            for jb in range(NJB):
                                sl = slice(jb * 512, (jb + 1) * 512)
                                ps = epsp.tile([128, 512], FP32, name=f"eps{i}_{jb}", tag="eps")
                                for k in range(NK):
                                    nc.tensor.matmul(ps, lhsT=qv[:, k, :],
                                                     rhs=kth_sb[k][:, sl],
                                                     start=(k == 0), stop=(k == NK - 1))
                                nc.vector.tensor_copy(e_sb[:, sl], ps)
                                nc.vector.reduce_max(mx8[:, jb:jb + 1], ps, axis=AX)
                                if jb < len(filler):
                                    filler[jb]()
                            return {"mx8": mx8, "e": e_sb}

                        def sm_exp(i, st):
                            """max-reduce + exp for i-tile (Act/DVE only, no PE)."""
                            mrow = smp.tile([128, 1], FP32, name=f"mrow{i}", tag="mrow")
                            nc.vector.reduce_max(mrow, st["mx8"], axis=AX)
                            negm = smp.tile([128, 1], FP32, name=f"negm{i}", tag="negm")
                            nc.vector.tensor_scalar_mul(negm, mrow, -SCALE)
                            p_sb = pp.tile([128, S], DF16, name=f"p{i}", tag="p")
                            lp8 = smp.tile([128, NJB], FP32, name=f"lp8_{i}", tag="lp8")
                            for jb in range(NJB):
                                sl = slice(jb * 512, (jb + 1) * 512)
                                nc.scalar.activation(
                                    p_sb[:, sl], st["e"][:, sl], Exp,
                                    bias=negm, scale=SCALE,
                                    accum_out=lp8[:, jb:jb + 1],
                                )
                            st["p"], st["lp8"] = p_sb, lp8

                        def sm_sum(i, st):
                            lrow = smp.tile([128, 1], FP32, name=f"lrow{i}", tag="lrow")
                            nc.vector.reduce_sum(lrow, st["lp8"], axis=AX)
                            linv = smp.tile([128, 1], FP32, name=f"linv{i}", tag="linv")
                            nc.vector.reciprocal(linv, lrow)
                            st["linv"] = linv

                        def sm_tp(i, st):
                            """P^T transposes for i-tile, 4 per PSUM tile so one
                            DVE copy moves 4 transposed blocks.  (An XBAR
                            dma_start_transpose was tried here and lost: its
                            ~7us DMA occupancy starves the prologue K/V loads
                            and the Y-filler Wo streams.)"""
                            p_sb = st["p"]
                            pt_sb = ptp.tile([128, NT, 128], DF16, name=f"pt{i}", tag="pt")
                            for g in range(NT // 4):
                                tp = tpsp.tile([128, 4, 128], DF16, name=f"tp{i}_{g}", tag="tp")
                                for w in range(4):
                                    t = g * 4 + w
                                    nc.tensor.transpose(
                                        tp[:, w, :], p_sb[:, t * 128:(t + 1) * 128], ident)
                                nc.vector.tensor_copy(pt_sb[:, g * 4:(g + 1) * 4], tp)
                            st["pt"] = pt_sb

                        def y_chunk(m, off, w):
                            # Y[m-rows, off:off+w] = sum_k Wo[k,m]^T @ O^T[k, cols]
                            if WO_RESIDENT:
                                wom = wo_sb[:, m]
                            else:
                                wom = wop.tile([128, NK, 128], DF16, name=f"wo{off}_{m}", tag="wom")
                                nc.sync.dma_start(wom, wo[m])
                            nsl = slice(off, off + w)
                            ps = ypsp.tile([128, 512], FP32, name=f"yps{m}_{off}", tag="yps")
                            for k in range(NK):
                                nc.tensor.matmul(
                                    ps[:, 0:w], lhsT=wom[:, k, :],
                                    rhs=ot_sb[k][:, nsl],
                                    start=(k == 0), stop=(k == NK - 1),
                                )
                            ys = ystp.tile([128, 512], DF16, name=f"ys{m}_{off}", tag="ys")
                            nc.vector.tensor_copy(ys[:, 0:w], ps[:, 0:w])
                            nc.sync.dma_start(yt[m * 128:(m + 1) * 128, nsl], ys[:, 0:w])

                        def y_chunks(off, w):
                            return [(lambda m=m: y_chunk(m, off, w)) for m in range(NK)]

                        def y_cols(off, w):
                            for f in y_chunks(off, w):
                                f()

                        def y_cols_packed(off, w, use_op_banks=False):
                            # 4 m-chunks accumulate into one PSUM bank (as 4
                            # separate groups), drained by a single copy --
                            # 1/4 the bank turnarounds for the exposed tail.
                            # After PV(7) the op0/op1 banks are idle: the final
                            # strip borrows them so its two groups never wait
                            # on each other's drain.
                            nsl = slice(off, off + w)
                            for mg in range(NK // 4):
                                if use_op_banks:
                                    ps = opsp.tile([128, 4, 128], FP32, name=f"yp4_{mg}_{off}",
                                                   tag=("op0", "op1")[mg % 2])
                                else:
                                    ps = ypsp.tile([128, 4, 128], FP32, name=f"yp4_{mg}_{off}",
                                                   tag="yps")
                                for w4 in range(4):
                                    m = mg * 4 + w4
                                    if WO_RESIDENT:
                                        wom = wo_sb[:, m]
                                    else:
                                        wom = wop.tile([128, NK, 128], DF16, name=f"wo{off}_{m}", tag="wom")
                                        nc.sync.dma_start(wom, wo[m])
                                    for k in range(NK):
                                        nc.tensor.matmul(
                                            ps[:, w4, 0:w], lhsT=wom[:, k, :],
                                            rhs=ot_sb[k][:, nsl],
                                            start=(k == 0), stop=(k == NK - 1),
                                        )
                                ys = ystp.tile([128, 4, 128], DF16, name=f"ys4_{mg}_{off}", tag="ys")
                                nc.vector.tensor_copy(ys, ps)
                                # one DMA for the whole 4-m row block
                                nc.sync.dma_start(
                                    yt[mg * 512:(mg + 1) * 512, nsl]
                                    .rearrange("(w p) f -> p w f", p=128),
                                    ys[:, :, 0:w])

                        def o_finish(i, op0, op1, linv):
                            """1/l scale + O transpose into ot_sb for one i-tile."""
                            isl = slice(i * 128, (i + 1) * 128)
                            osb = obp.tile([128, D], DF16, name=f"osb{i}", tag="osb")
                            nc.vector.tensor_scalar_mul(osb[:, 0:512], op0, linv)
                            nc.vector.tensor_scalar_mul(osb[:, 512:D], op1, linv)
                            for g in range(NK // 4):
                                tp = tpsp.tile([128, 4, 128], DF16, name=f"otp{i}_{g}", tag="tp")
                                for w in range(4):
                                    t = g * 4 + w
                                    nc.tensor.transpose(
                                        tp[:, w, :], osb[:, t * 128:(t + 1) * 128], ident)
                                for w in range(4):
                                    nc.vector.tensor_copy(
                                        ot_sb[g * 4 + w][:, isl], tp[:, w, :])

                        if True:
                            # software pipeline: E(i+1) is emitted between
                            # softmax(i)'s Act/DVE chain and the P^T/PV(i) PE
                            # work, so the PE never idles waiting on softmax
                            # Y strips ride along as E-sweep fillers once their
                            # o_finish dependencies are met: E(i+1) is emitted in
                            # iteration i, so strip cols [c, c+256) (i-tiles
                            # c/128..c/128+1) can fill E(c/128+2)'s sweep
                            fillers = {5: y_chunks(0, 256), 6: y_chunks(256, 256),
                                       7: y_chunks(512, 256)}
                            st = {0: e_sweep(0)}
                            for i in range(NI):
                                s = st.pop(i)
                                sm_exp(i, s)
                                if i + 1 < NI:
                                    st[i + 1] = e_sweep(i + 1, fillers.get(i + 1, ()))
                                else:
                                    # no E to fill the last softmax window --
                                    # use the last even-numbered strip instead
                                    y_cols_packed(768, 128)
                                sm_sum(i, s)
                                sm_tp(i, s)
                                pt_sb = s["pt"]
                                op0 = opsp.tile([128, 512], FP32, name=f"op0_{i}", tag="op0")
                                op1 = opsp.tile([128, 512], FP32, name=f"op1_{i}", tag="op1")
                                for t in range(NT):
                                    nc.tensor.matmul(
                                        op0, lhsT=pt_sb[:, t, :], rhs=vres[:, t, 0:512],
                                        start=(t == 0), stop=(t == NT - 1))
                                    nc.tensor.matmul(
                                        op1, lhsT=pt_sb[:, t, :], rhs=vres[:, t, 512:D],
                                        start=(t == 0), stop=(t == NT - 1))
                                o_finish(i, op0, op1, s["linv"])
                            y_cols_packed(896, 128, use_op_banks=True)
    nc.compile()
    return nc


def _get_programs():
    if "nc1" not in _cache:
        _cache["nc1"] = _build_phase1()
        _cache["nc2"] = _build_phase2()
    return _cache["nc1"], _cache["nc2"]


def kernel(x, Wq, Wk, Wv, Wo):
    from concourse.bass_utils import run_bass_kernel_spmd

    nc1, nc2 = _get_programs()

    x = np.asarray(x, dtype=np.float32)
    wq_h = np.asarray(Wq, dtype=np.float32).astype(F16)
    wk_h = np.asarray(Wk, dtype=np.float32).astype(F16)
    wo_h = np.asarray(Wo, dtype=np.float32).astype(F16)
    wo_blk = np.ascontiguousarray(
        wo_h.reshape(NK, 128, NK, 128).transpose(2, 1, 0, 3))
    wvh8, wvl8 = _split_e4m3(np.asarray(Wv, dtype=np.float32))
    # [p, n, r, d] with slots (hi, lo*32)
    wv8 = np.ascontiguousarray(
        np.stack([wvh8, wvl8], axis=0)                  # [r, (n p), d]
        .reshape(2, NK, 128, D).transpose(2, 1, 0, 3))

    # ---- phase 1: per-core row slices ----
    in1 = []
    for c in range(8):
        b, i = divmod(c, 4)
        rows = x[b, i * BLK:(i + 1) * BLK, :]           # [BLK, D]
        xt32 = np.ascontiguousarray(rows.T)             # [D, BLK] fp32
        xh8, xl8 = _split_e4m3(xt32)
        x8 = np.ascontiguousarray(
            np.stack([xh8, xl8], axis=0)                # [r, (n p), s]
            .reshape(2, NK, 128, BLK).transpose(2, 1, 0, 3))
        in1.append({
            "xt": xt32.astype(F16), "x8": x8,
            "wq": wq_h, "wk": wk_h, "wv8": wv8,
        })
    res1 = run_bass_kernel_spmd(nc1, in1, list(range(8))).results

    # ---- host gather of K/V shards into per-batch tensors ----
    kth_full, v_full = [], []
    for b in range(B):
        kth_full.append(np.concatenate(
            [res1[b * 4 + i]["kt"] for i in range(4)], axis=1))    # [D, S]
        v_full.append(np.concatenate(
            [res1[b * 4 + i]["vo"] for i in range(4)], axis=0))    # [NT, 128, D]

    # ---- phase 2 ----
    in2 = []
    for c in range(8):
        b, i = divmod(c, 4)
        qt_c = res1[c]["qt"]                                     # [D, BLK] fp16
        # [n, p, i, f] -> [i, p, n, f]
        qt1_c = np.ascontiguousarray(
            qt_c.reshape(NK, 128, NI, 128).transpose(2, 1, 0, 3))
        in2.append({
            "kth": kth_full[b], "vin": v_full[b],
            "qt1": qt1_c,
            "wo": wo_blk,
        })
    res2 = run_bass_kernel_spmd(nc2, in2, list(range(8))).results

    out = np.empty((B, S, D), dtype=np.float32)
    for c in range(8):
        b, i = divmod(c, 4)
        out[b, i * BLK:(i + 1) * BLK, :] = res2[c]["yt"].T
    return out


# revision 107
# speedup vs baseline: 1.0235x; 1.0235x over previous
"""Trainium2 Bass kernel for single-head self-attention (B=2, S=4096, D=1024).

reference:
    q = x @ Wq; k = x @ Wk; v = x @ Wv          # [B,S,D]
    energy = einsum('bid,bjd->bij', q, k) * 8.0  # SCALE = sqrt(64)
    attn = softmax(energy, axis=-1)
    out = einsum('bij,bjd->bid', attn, v) @ Wo

Two SPMD launches over 8 cores (= 2 batches x 4 query-blocks of 1024):
  phase 1: each core computes the Q/K/V projections for its own 1024
           rows only (1/8 of the total work, no redundancy); the host
           gathers K/V shards into full per-batch tensors.
  phase 2: each core runs attention + output projection for its block
           against the full K/V of its batch.

Precision: logits have std ~256 (SCALE multiplies), so softmax is
nearly an argmax -- the x->Q, x->K, Q@K^T path needs ~2^-12 relative
accuracy.  fp16 (e5m10) gives 2^-11.8 rounding at the same 1
cycle/row PE rate as bf16, and Q/K values are ~N(0,1) so the e5
range is ample; those matmuls run as a single fp16 pass with fp32
PSUM accumulation (end-to-end rel err ~8e-3, dominated by the
softmax's sensitivity to logit rounding; bf16 single-pass would be
~5e-2, and the old bf16 hi/lo 3-pass scheme costs 3x the cycles).
The V projection tolerates ~0.5% error, so it runs entirely as fp8
e4m3 DoubleRow matmuls (0.5 cycles/row, two k-tiles contracted per
instruction = 4x bf16 MAC rate in the cost model): hi*hi plus the
two hi*lo cross terms, with the lo planes pre-scaled by 32 to clear
the e4m3 denormal floor and the 1/32 folded back during the
PSUM->SBUF merge (scalar-engine Copy-with-scale + DVE add).

Phase 2 is software-pipelined: each iteration emits softmax(i)'s
Act/DVE chain, then E(i+1)'s matmuls (which keep the PE busy through
the softmax latency), then P^T transposes and P@V for i.  K, V and
the full output projection Wo path are scheduled so V stays resident
in SBUF all phase (the fp16/fp8 shrink of K freed the room), and the
output projection runs as column strips woven into the E sweeps'
PSUM-drain slots, with the last strip packed 4-chunks-per-bank into
the by-then-idle PV banks.

Layout: feature-major ("transposed") activations throughout; the host
pre-transposes x and post-transposes the output. DMA instruction count
is kept low (batched loads/stores) -- each HWDGE descriptor-generation
costs ~0.65us of serialized queue-prep time on its issuing engine, and
all transfers serialize on the DMA-engine pool at ~360 GB/s, so issue
order is chosen to match consumption order (prologue loads split
across the SP and Activation HWDGE queues).
"""

import numpy as np

B, S, D = 2, 4096, 1024
BLK = 1024          # queries per core
SCALE = 8.0         # HEAD_DIM ** 0.5 = sqrt(64)
NK = D // 128       # 8 k-tiles over the feature dim
NT = S // 128       # 32 j-tiles over keys
NI = BLK // 128     # 8 i-tiles over this core's queries
NJB = S // 512      # 8 key blocks of 512
F16 = np.float16
import ml_dtypes
E4M3 = ml_dtypes.float8_e4m3
LO_SCALE = np.float32(32.0)   # pre-scale on e4m3 lo planes (kept out of denormals)


def _split_e4m3(a):
    """fp32 -> (hi, lo) e4m3 planes with lo pre-scaled by LO_SCALE."""
    hi = a.astype(E4M3)
    lo = ((a - hi.astype(np.float32)) * LO_SCALE).astype(E4M3)
    return hi, lo

# phase-2 tuning knobs (swept via TimelineSim); PSUM is 8 banks total:
# EPS + TP + 2 (op0/op1) + YPS <= 8
EPS_BUFS = 3        # PSUM banks for E accumulation
TP_BUFS = 2         # PSUM banks for PE transposes
YPS_BUFS = 1        # PSUM banks for Y (out-projection) accumulation
DUAL_Q = True       # split prologue DMA issue across SP + Act HWDGE queues
FIRST_BAND_512 = False  # 512-col leading K bands instead of 1024
WO_RESIDENT = False  # keep all of Wo in SBUF instead of streaming per strip
QV23 = True         # preload qv2/qv3 on the Act queue
STRIP_MID = True    # Y strips ride as E-sweep fillers (see `fillers`)

_cache = {}


def _build_phase1():
    """Q/K/V projections for this core's 1024 rows."""
    import concourse.mybir as mybir
    from concourse import bacc
    from concourse.tile import TileContext

    FP32 = mybir.dt.float32
    DF16 = mybir.dt.float16
    DF8 = mybir.dt.float8e4
    DR = mybir.MatmulPerfMode.DoubleRow
    ADD = mybir.AluOpType.add
    Copy = mybir.ActivationFunctionType.Copy

    nc = bacc.Bacc("TRN2", target_bir_lowering=False, debug=False, num_devices=8)

    xt = nc.dram_tensor("xt", [D, BLK], DF16, kind="ExternalInput")   # rows.T
    # e4m3 2-level x/Wv for the V projection; lo planes pre-scaled by 32
    x8 = nc.dram_tensor("x8", [128, NK, 2, BLK], DF8, kind="ExternalInput")
    wv8 = nc.dram_tensor("wv8", [128, NK, 2, D], DF8, kind="ExternalInput")
    wq = nc.dram_tensor("wq", [D, D], DF16, kind="ExternalInput")
    wk = nc.dram_tensor("wk", [D, D], DF16, kind="ExternalInput")
    qt = nc.dram_tensor("qt", [D, BLK], DF16, kind="ExternalOutput")
    kt = nc.dram_tensor("kt", [D, BLK], DF16, kind="ExternalOutput")
    vo = nc.dram_tensor("vo", [NI, 128, D], DF16, kind="ExternalOutput")

    with TileContext(nc) as tc:
      with (
          tc.tile_pool(name="xqk", bufs=1) as xqkp,
          tc.tile_pool(name="wkqp", bufs=1) as wkqp,
          tc.tile_pool(name="wvp", bufs=1) as wvp,
          tc.tile_pool(name="vps", bufs=2, space="PSUM") as vps,
          tc.tile_pool(name="vsb", bufs=3) as vsbp,
      ):
        if True:
            # load order = consumption order; first K-weight sliver and x
            # halves first so the K sweep starts as early as possible
            xt_r = xt[:, :].rearrange("(n p) s -> p n s", p=128)
            xs = xqkp.tile([128, NK, BLK], DF16, name="xs", tag="xs")
            w_sb = {}
            wk_sb = wkqp.tile([128, NK, D], DF16, name="kwt", tag="kwt")
            wk_r = wk[:, :].rearrange("(n p) d -> p n d", p=128)
            nc.scalar.dma_start(wk_sb[:, :, 0:128], wk_r[:, :, 0:128])
            nc.sync.dma_start(xs[:, :, 0:128], xt_r[:, :, 0:128])
            nc.sync.dma_start(xs[:, :, 128:256], xt_r[:, :, 128:256])
            nc.scalar.dma_start(wk_sb[:, :, 128:512], wk_r[:, :, 128:512])
            nc.sync.dma_start(xs[:, :, 256:512], xt_r[:, :, 256:512])
            nc.sync.dma_start(xs[:, :, 512:BLK], xt_r[:, :, 512:BLK])
            nc.scalar.dma_start(wk_sb[:, :, 512:D], wk_r[:, :, 512:D])
            w_sb["k"] = wk_sb
            x8_sb = wvp.tile([128, NK, 2, BLK], DF8, name="x8t", tag="x8t")
            nc.scalar.dma_start(x8_sb, x8[:, :, :, :])
            wv8_sb = wvp.tile([128, NK, 2, D], DF8, name="wv8t", tag="wv8t")
            nc.sync.dma_start(wv8_sb, wv8[:, :, :, :])
            wq_sb = wkqp.tile([128, NK, D], DF16, name="qwt", tag="qwt")
            wq_r = wq[:, :].rearrange("(n p) d -> p n d", p=128)
            nc.scalar.dma_start(wq_sb[:, :, 0:512], wq_r[:, :, 0:512])
            nc.sync.dma_start(wq_sb[:, :, 512:D], wq_r[:, :, 512:D])
            w_sb["q"] = wq_sb
        # ------------- Kt, then V, then Qt -------------
        if True:
            def v_section():
                # V = x @ Wv in 2-level e4m3: hi*hi into one PSUM bank, the
                # two cross terms (carrying the x32 lo pre-scale) into a
                # second, merged by one (cross/32 + main) DVE op.  All four
                # products run as fp8 DoubleRow matmuls (0.5 cycles/row,
                # k-tile pairs), 0.75x the fp16 cost.
                NH = NK // 2
                for j in range(NI):
                    vt = vsbp.tile([128, D], DF16, name=f"vt{j}", tag="vt")
                    jsl = slice(j * 128, (j + 1) * 128)
                    for db in range(2):
                        dsl = slice(db * 512, (db + 1) * 512)
                        psm = vps.tile([128, 512], FP32, name=f"vpm{j}_{db}", tag="vpm")
                        psc = vps.tile([128, 512], FP32, name=f"vpc{j}_{db}", tag="vpc")
                        for kp in range(NH):
                            k2 = slice(2 * kp, 2 * kp + 2)
                            nc.tensor.matmul(
                                psm, lhsT=x8_sb[:, k2, 0, jsl],
                                rhs=wv8_sb[:, k2, 0, dsl], perf_mode=DR,
                                start=(kp == 0), stop=(kp == NH - 1))
                        for kp in range(NH):
                            k2 = slice(2 * kp, 2 * kp + 2)
                            nc.tensor.matmul(
                                psc, lhsT=x8_sb[:, k2, 0, jsl],
                                rhs=wv8_sb[:, k2, 1, dsl], perf_mode=DR,
                                start=(kp == 0), stop=False)
                            nc.tensor.matmul(
                                psc, lhsT=x8_sb[:, k2, 1, jsl],
                                rhs=wv8_sb[:, k2, 0, dsl], perf_mode=DR,
                                start=False, stop=(kp == NH - 1))
                        # cross/32 then add, both on DVE (no cross-engine
                        # handshake; walrus rejects dual-PSUM fused forms)
                        vc = vsbp.tile([128, 512], DF16, name=f"vc{j}_{db}", tag="vc")
                        nc.vector.tensor_scalar_mul(vc, psc, float(1.0 / LO_SCALE))
                        nc.vector.tensor_tensor(vt[:, dsl], psm, vc, op=ADD)
                    nc.sync.dma_start(vo[j], vt)

            # K's leading column-chunks are narrow so its first PSUM group
            # only needs 128 columns of x to have landed
            CHUNKS = {"k": ((0, 128), (128, 256), (256, 384), (384, 512), (512, 1024)),
                      "q": ((0, 512), (512, 1024))}
            for (outd, pfx) in ((kt, "k"), (qt, "q")):
                with (
                    tc.tile_pool(name=f"{pfx}ps", bufs=4, space="PSUM") as psp,
                    tc.tile_pool(name=f"{pfx}st", bufs=1) as stp,
                ):
                    w_t = w_sb[pfx]
                    sh_t = []
                    for m in range(NK):
                        sh_t.append(stp.tile([128, BLK], DF16, name=f"{pfx}sh{m}", tag=f"sh{m}"))
                    chunks = CHUNKS[pfx]
                    lc = len(chunks) - 1
                    if pfx == "k":
                        # ordered by operand arrival: narrow x chunks against
                        # the first weight columns first, high-m columns once
                        # the rest of the weight lands, then the wide chunk
                        order = ([(ci, m) for ci in range(4) for m in range(4)]
                                 + [(ci, m) for ci in range(4) for m in range(4, NK)]
                                 + [(lc, m) for m in range(NK)])
                    else:
                        order = [(ci, m) for ci in range(len(chunks)) for m in range(NK)]
                    for (ci, m) in order:
                        c0, c1 = chunks[ci]
                        nsl = slice(c0, c1)
                        msl = slice(m * 128, (m + 1) * 128)
                        ps = psp.tile([128, 512], FP32, name=f"{pfx}ps{ci}_{m}", tag="ps")
                        for k in range(NK):
                            nc.tensor.matmul(ps[:, 0:c1 - c0], lhsT=w_t[:, k, msl],
                                             rhs=xs[:, k, nsl],
                                             start=(k == 0), stop=(k == NK - 1))
                        nc.vector.tensor_copy(sh_t[m][:, nsl], ps[:, 0:c1 - c0])
                        if ci == lc:
                            # store as soon as the last chunk of m is done
                            nc.sync.dma_start(outd[msl, :], sh_t[m])
                if pfx == "k":
                    v_section()
    nc.compile()
    return nc


def _build_phase2():
    """Attention + output projection for this core's 1024 queries."""
    import concourse.mybir as mybir
    from concourse import bacc
    from concourse.tile import TileContext
    from concourse.masks import make_identity

    FP32 = mybir.dt.float32
    DF16 = mybir.dt.float16
    DF8 = mybir.dt.float8e4
    DR = mybir.MatmulPerfMode.DoubleRow
    ADD = mybir.AluOpType.add
    SUB = mybir.AluOpType.subtract
    Exp = mybir.ActivationFunctionType.Exp
    Copy = mybir.ActivationFunctionType.Copy
    AX = mybir.AxisListType.X

    nc = bacc.Bacc("TRN2", target_bir_lowering=False, debug=False, num_devices=8)

    kth = nc.dram_tensor("kth", [D, S], DF16, kind="ExternalInput")
    # per-i-tile partition-major Q: [i, p, n, f] = qt[n*128+p, i*128+f]
    qt1 = nc.dram_tensor("qt1", [NI, 128, NK, 128], DF16, kind="ExternalInput")
    vin = nc.dram_tensor("vin", [NT, 128, D], DF16, kind="ExternalInput")
    # 2-plane e4m3 Wo, lo plane pre-scaled by 32:
    # [m, p, r, n, f] = plane_r[n*128+p, m*128+f]
    wo8 = nc.dram_tensor("wo8", [NK, 128, 2, NK, 128], DF8, kind="ExternalInput")
    # fp16 Wo for the two exposed final strips
    wo16 = nc.dram_tensor("wo16", [NK, 128, NK, 128], DF16, kind="ExternalInput")
    yt = nc.dram_tensor("yt", [D, BLK], DF16, kind="ExternalOutput")

    from contextlib import ExitStack
    with TileContext(nc) as tc:
        with ExitStack() as stack:
            constp = stack.enter_context(tc.tile_pool(name="const", bufs=1))
            ident = constp.tile([128, 128], DF16)
            make_identity(nc, ident)

            ktp = stack.enter_context(tc.tile_pool(name="ktp", bufs=1))
            qtp = stack.enter_context(tc.tile_pool(name="qtp", bufs=3))
            if True:
                # first two i-tiles' Q loads before the big K transfers
                qv0 = qtp.tile([128, NK, 128], DF16, name="qv0", tag="qv")
                nc.sync.dma_start(qv0, qt1[0])
                qv1 = qtp.tile([128, NK, 128], DF16, name="qv1", tag="qv")
                nc.sync.dma_start(qv1, qt1[1])
                # column-banded K loads, issued on BOTH HWDGE queues (SP +
                # Activation) to halve the ~0.65us/DMA descriptor-gen
                # serialization in the prologue; V (resident all phase) and
                # Wo (resident) follow
                kth_sb = []
                for m in range(NK):
                    kth_sb.append(ktp.tile([128, S], DF16, name=f"kth{m}", tag=f"kth{m}"))
                vrp = stack.enter_context(tc.tile_pool(name="vrp", bufs=1))
                vres = vrp.tile([128, NT, D], DF16, name="vres", tag="vres")
                vin_r = vin[:, :, :].rearrange("t p d -> p t d")
                if FIRST_BAND_512:
                    bands = ((0, 512), (512, 1024), (1024, 2048),
                             (2048, 3072), (3072, 4096))
                else:
                    bands = ((0, 1024), (1024, 2048),
                             (2048, 3072), (3072, 4096))
                for bi, (c0, c1) in enumerate(bands):
                    csl = slice(c0, c1)
                    for m in range(NK):
                        eng = nc.scalar if (DUAL_Q and bi < 2 and m >= NK // 2) else nc.sync
                        eng.dma_start(kth_sb[m][:, csl], kth[m * 128:(m + 1) * 128, csl])
                for cb in range(4):
                    tsl = slice(cb * (NT // 4), (cb + 1) * (NT // 4))
                    nc.sync.dma_start(vres[:, tsl], vin_r[:, tsl])
                # everything below is needed only from the middle of the
                # phase on -- keep it BEHIND the V quarters in the DMA FIFO
                qv23 = []
                if QV23:
                    for i in (2, 3):
                        qvn = qtp.tile([128, NK, 128], DF16, name=f"qv{i}", tag="qv")
                        (nc.scalar if DUAL_Q else nc.sync).dma_start(qvn, qt1[i])
                        qv23.append(qvn)
                wop = stack.enter_context(tc.tile_pool(name="wop", bufs=2))

                otp = stack.enter_context(tc.tile_pool(name="otp", bufs=1))
                if True:
                    # O^T in 2-plane e4m3 (lo plane pre-scaled by 32) so the
                    # output projection runs as fp8 DoubleRow matmuls; the
                    # last i-tile's columns stay fp16 (see o_finish)
                    ot8h = otp.tile([128, NK, 768], DF8, name="ot8h", tag="ot8h")
                    ot8l = otp.tile([128, NK, 768], DF8, name="ot8l", tag="ot8l")
                    # fp16 O^T for the last two i-tiles' columns (896:1024 and
                    # 768:896) -- they only feed the two exposed fp16 strips
                    ot16t = otp.tile([128, NK, 256], DF16, name="ot16t", tag="ot16t")
                    epsp = stack.enter_context(tc.tile_pool(name="eps", bufs=EPS_BUFS, space="PSUM"))
                    tpsp = stack.enter_context(tc.tile_pool(name="tps", bufs=TP_BUFS, space="PSUM"))
                    opsp = stack.enter_context(tc.tile_pool(name="ops", bufs=1, space="PSUM"))
                    smp = stack.enter_context(tc.tile_pool(name="smp", bufs=2))
                    esp = stack.enter_context(tc.tile_pool(name="esp", bufs=2))
                    pp = stack.enter_context(tc.tile_pool(name="pp", bufs=1))
                    ptp = stack.enter_context(tc.tile_pool(name="ptp", bufs=1))
                    obp = stack.enter_context(tc.tile_pool(name="obp", bufs=1))
                    ypsp = stack.enter_context(tc.tile_pool(name="yps", bufs=YPS_BUFS, space="PSUM"))
                    ystp = stack.enter_context(tc.tile_pool(name="yst", bufs=1))
                    if True:
                        qvt = {0: qv0, 1: qv1}
                        for j, t in enumerate(qv23):
                            qvt[2 + j] = t

                        def get_qv(i):
                            if i not in qvt:
                                qvt[i] = qtp.tile([128, NK, 128], DF16, name=f"qv{i}", tag="qv")
                                nc.sync.dma_start(qvt[i], qt1[i])
                            return qvt[i]

                        def e_sweep(i, filler=()):
                            """E = Q_i @ K^T for one i-tile (raw logits + per-block
                            maxes).  One `filler` callable runs after each j-block's
                            matmuls -- deferred PE work (Y chunks) slots in while
                            the block's PSUM drains, hiding its bank turnaround."""
                            qv = get_qv(i)
                            if i + 1 < NI:
                                get_qv(i + 1)  # prefetch next Q tile
                            mx8 = smp.tile([128, NJB], FP32, name=f"mx8_{i}", tag="mx8")
                            e_sb = esp.tile([128, S], FP32, name=f"e{i}", tag="e")
                            for jb in range(NJB):
                                sl = slice(jb * 512, (jb + 1) * 512)
                                ps = epsp.tile([128, 512], FP32, name=f"eps{i}_{jb}", tag="eps")
                                for k in range(NK):
                                    nc.tensor.matmul(ps, lhsT=qv[:, k, :],
                                                     rhs=kth_sb[k][:, sl],
                                                     start=(k == 0), stop=(k == NK - 1))
                                nc.vector.tensor_copy(e_sb[:, sl], ps)
                                nc.vector.reduce_max(mx8[:, jb:jb + 1], ps, axis=AX)
                                if jb < len(filler):
                                    filler[jb]()
                            return {"mx8": mx8, "e": e_sb}

                        def sm_exp(i, st):
                            """max-reduce + exp for i-tile (Act/DVE only, no PE)."""
                            mrow = smp.tile([128, 1], FP32, name=f"mrow{i}", tag="mrow")
                            nc.vector.reduce_max(mrow, st["mx8"], axis=AX)
                            negm = smp.tile([128, 1], FP32, name=f"negm{i}", tag="negm")
                            nc.vector.tensor_scalar_mul(negm, mrow, -SCALE)
                            p_sb = pp.tile([128, S], DF16, name=f"p{i}", tag="p")
                            lp8 = smp.tile([128, NJB], FP32, name=f"lp8_{i}", tag="lp8")
                            for jb in range(NJB):
                                sl = slice(jb * 512, (jb + 1) * 512)
                                nc.scalar.activation(
                                    p_sb[:, sl], st["e"][:, sl], Exp,
                                    bias=negm, scale=SCALE,
                                    accum_out=lp8[:, jb:jb + 1],
                                )
                            st["p"], st["lp8"] = p_sb, lp8

                        def sm_sum(i, st):
                            lrow = smp.tile([128, 1], FP32, name=f"lrow{i}", tag="lrow")
                            nc.vector.reduce_sum(lrow, st["lp8"], axis=AX)
                            linv = smp.tile([128, 1], FP32, name=f"linv{i}", tag="linv")
                            nc.vector.reciprocal(linv, lrow)
                            st["linv"] = linv

                        def sm_tp(i, st):
                            """P^T transposes for i-tile, 4 per PSUM tile so one
                            DVE copy moves 4 transposed blocks.  (An XBAR
                            dma_start_transpose was tried here and lost: its
                            ~7us DMA occupancy starves the prologue K/V loads
                            and the Y-filler Wo streams.)"""
                            p_sb = st["p"]
                            pt_sb = ptp.tile([128, NT, 128], DF16, name=f"pt{i}", tag="pt")
                            for g in range(NT // 4):
                                tp = tpsp.tile([128, 4, 128], DF16, name=f"tp{i}_{g}", tag="tp")
                                for w in range(4):
                                    t = g * 4 + w
                                    nc.tensor.transpose(
                                        tp[:, w, :], p_sb[:, t * 128:(t + 1) * 128], ident)
                                nc.vector.tensor_copy(pt_sb[:, g * 4:(g + 1) * 4], tp)
                            st["pt"] = pt_sb

                        NH = NK // 2

                        def y_mms(ps_main, ps_cross, wom, nsl, first, last):
                            # main (hi*hi) and the two cross terms (carrying the
                            # x32 lo pre-scale) as fp8 DoubleRow k-pair matmuls
                            for kp in range(NH):
                                k2 = slice(2 * kp, 2 * kp + 2)
                                nc.tensor.matmul(
                                    ps_main, lhsT=wom[:, 0, k2, :],
                                    rhs=ot8h[:, k2, nsl], perf_mode=DR,
                                    start=first and kp == 0, stop=last and kp == NH - 1)
                            for kp in range(NH):
                                k2 = slice(2 * kp, 2 * kp + 2)
                                nc.tensor.matmul(
                                    ps_cross, lhsT=wom[:, 1, k2, :],
                                    rhs=ot8h[:, k2, nsl], perf_mode=DR,
                                    start=first and kp == 0, stop=False)
                                nc.tensor.matmul(
                                    ps_cross, lhsT=wom[:, 0, k2, :],
                                    rhs=ot8l[:, k2, nsl], perf_mode=DR,
                                    start=False, stop=last and kp == NH - 1)

                        def y_chunk(m, off, w):
                            # Y[m-rows, off:off+w] = sum_k Wo[k,m]^T @ O^T[k, cols]
                            # main group in the low half of the bank, cross
                            # group in the high half; merged by Act(1/32) + add
                            wom = wop.tile([128, 2, NK, 128], DF8, name=f"wo{off}_{m}", tag="wom")
                            nc.sync.dma_start(wom, wo8[m])
                            nsl = slice(off, off + w)
                            ps = ypsp.tile([128, 512], FP32, name=f"yps{m}_{off}", tag="yps")
                            y_mms(ps[:, 0:w], ps[:, 256:256 + w], wom, nsl, True, True)
                            yc = ystp.tile([128, 256], DF16, name=f"yc{m}_{off}", tag="yc")
                            nc.vector.tensor_scalar_mul(yc[:, 0:w], ps[:, 256:256 + w],
                                                        float(1.0 / LO_SCALE))
                            ys = ystp.tile([128, 256], DF16, name=f"ys{m}_{off}", tag="ys")
                            nc.vector.tensor_tensor(ys[:, 0:w], ps[:, 0:w], yc[:, 0:w], op=ADD)
                            nc.sync.dma_start(yt[m * 128:(m + 1) * 128, nsl], ys[:, 0:w])

                        def y_chunks(off, w):
                            return [(lambda m=m: y_chunk(m, off, w)) for m in range(NK)]

                        def y_cols_packed(off, w, tags=("yps",)):
                            # 2 m-chunks (main+cross each) per PSUM bank; a
                            # ring of banks hides the drain turnarounds of an
                            # exposed strip (banks not named "yps" borrow the
                            # PV op0/op1 banks, which are free between
                            # o_finish(i-1) and PV(i)).
                            assert w <= 128
                            nsl = slice(off, off + w)
                            for mg in range(NK // 2):
                                tag = tags[mg % len(tags)]
                                pool = ypsp if tag == "yps" else opsp
                                ps = pool.tile([128, 4, 128], FP32, name=f"yp4_{mg}_{off}",
                                               tag=tag)
                                for w2 in range(2):
                                    m = mg * 2 + w2
                                    wom = wop.tile([128, 2, NK, 128], DF8,
                                                   name=f"wo{off}_{m}", tag="wom")
                                    nc.sync.dma_start(wom, wo8[m])
                                    y_mms(ps[:, 2 * w2, 0:w], ps[:, 2 * w2 + 1, 0:w],
                                          wom, nsl, True, True)
                                yc = ystp.tile([128, 2, 128], DF16, name=f"yc4_{mg}_{off}", tag="yc")
                                nc.vector.tensor_scalar_mul(yc[:, :, 0:w], ps[:, 1::2, 0:w],
                                                            float(1.0 / LO_SCALE))
                                ys = ystp.tile([128, 2, 128], DF16, name=f"ys4_{mg}_{off}", tag="ys4")
                                nc.vector.tensor_tensor(ys[:, :, 0:w], ps[:, 0::2, 0:w],
                                                        yc[:, :, 0:w], op=ADD)
                                # one DMA for the whole 2-m row block
                                nc.sync.dma_start(
                                    yt[mg * 256:(mg + 1) * 256, nsl]
                                    .rearrange("(w p) f -> p w f", p=128),
                                    ys[:, :, 0:w])

                        def y_cols_f16(off, tags):
                            # exposed final strips in fp16: longer per-chunk
                            # compute hides the bank drains; 4 chunks per bank,
                            # one copy + one batched store per 4-m block.  The
                            # fp16 Wo tiles borrow the idle qv pool buffers.
                            nsl = slice(off, off + 128)
                            csl = slice(off - 768, off - 768 + 128)
                            for mg in range(NK // 4):
                                tag = tags[mg % len(tags)]
                                pool = ypsp if tag == "yps" else opsp
                                ps = pool.tile([128, 4, 128], FP32, name=f"yf{mg}_{off}", tag=tag)
                                for w4 in range(4):
                                    m = mg * 4 + w4
                                    wom = qtp.tile([128, NK, 128], DF16,
                                                   name=f"wo16_{off}_{m}", tag="qv")
                                    nc.sync.dma_start(wom, wo16[m])
                                    for k in range(NK):
                                        nc.tensor.matmul(
                                            ps[:, w4, :], lhsT=wom[:, k, :],
                                            rhs=ot16t[:, k, csl],
                                            start=(k == 0), stop=(k == NK - 1),
                                        )
                                ys = ystp.tile([128, 4, 128], DF16, name=f"yf16_{mg}_{off}", tag="ys4")
                                nc.vector.tensor_copy(ys, ps)
                                nc.sync.dma_start(
                                    yt[mg * 512:(mg + 1) * 512, nsl]
                                    .rearrange("(w p) f -> p w f", p=128),
                                    ys)

                        def o_finish(i, op0, op1, linv):
                            """1/l scale + O transpose for one i-tile.  i<7
                            feeds the 2-plane e4m3 O^T tensors (its columns are
                            consumed by fp8 Y strips mid-phase); the last
                            i-tile keeps a short fp16 path since its columns
                            only feed the exposed final strip, where the e4m3
                            split chain would sit on the critical tail."""
                            isl = slice(i * 128, (i + 1) * 128)
                            osb = obp.tile([128, D], DF16, name=f"osb{i}", tag="osb")
                            nc.vector.tensor_scalar_mul(osb[:, 0:512], op0, linv)
                            nc.vector.tensor_scalar_mul(osb[:, 512:D], op1, linv)
                            for g in range(NK // 4):
                                g4 = slice(g * 4, (g + 1) * 4)
                                tp = tpsp.tile([128, 4, 128], DF16, name=f"otp{i}_{g}", tag="tp")
                                for w in range(4):
                                    t = g * 4 + w
                                    nc.tensor.transpose(
                                        tp[:, w, :], osb[:, t * 128:(t + 1) * 128], ident)
                                if i >= NI - 2:
                                    t16 = slice((i - (NI - 2)) * 128, (i - (NI - 2)) * 128 + 128)
                                    nc.vector.tensor_copy(ot16t[:, g4, t16], tp)
                                else:
                                    nc.vector.tensor_copy(ot8h[:, g4, isl], tp)
                                    tmp = obp.tile([128, 4, 128], DF16, name=f"otmp{i}_{g}", tag="otmp")
                                    nc.vector.tensor_tensor(tmp, tp, ot8h[:, g4, isl], op=SUB)
                                    nc.scalar.activation(ot8l[:, g4, isl], tmp, Copy,
                                                         scale=float(LO_SCALE))

                        if True:
                            # software pipeline: E(i+1) is emitted between
                            # softmax(i)'s Act/DVE chain and the P^T/PV(i) PE
                            # work, so the PE never idles waiting on softmax
                            # Y strips ride along as E-sweep fillers once their
                            # o_finish dependencies are met: E(i+1) is emitted in
                            # iteration i, so strip cols [c, c+256) (i-tiles
                            # c/128..c/128+1) can fill E(c/128+2)'s sweep
                            fillers = {5: y_chunks(0, 256), 6: y_chunks(256, 256),
                                       7: y_chunks(512, 256)}
                            st = {0: e_sweep(0)}
                            for i in range(NI):
                                s = st.pop(i)
                                sm_exp(i, s)
                                if i + 1 < NI:
                                    st[i + 1] = e_sweep(i + 1, fillers.get(i + 1, ()))
                                else:
                                    # no E to fill the last softmax window --
                                    # use the last even-numbered strip instead
                                    y_cols_f16(768, tags=("yps",))
                                sm_sum(i, s)
                                sm_tp(i, s)
                                pt_sb = s["pt"]
                                op0 = opsp.tile([128, 512], FP32, name=f"op0_{i}", tag="op0")
                                op1 = opsp.tile([128, 512], FP32, name=f"op1_{i}", tag="op1")
                                for t in range(NT):
                                    nc.tensor.matmul(
                                        op0, lhsT=pt_sb[:, t, :], rhs=vres[:, t, 0:512],
                                        start=(t == 0), stop=(t == NT - 1))
                                    nc.tensor.matmul(
                                        op1, lhsT=pt_sb[:, t, :], rhs=vres[:, t, 512:D],
                                        start=(t == 0), stop=(t == NT - 1))
                                o_finish(i, op0, op1, s["linv"])
                            y_cols_f16(896, tags=("op0", "op1"))
    nc.compile()
    return nc


def _get_programs():
    if "nc1" not in _cache:
        _cache["nc1"] = _build_phase1()
        _cache["nc2"] = _build_phase2()
    return _cache["nc1"], _cache["nc2"]


def kernel(x, Wq, Wk, Wv, Wo):
    from concourse.bass_utils import run_bass_kernel_spmd

    nc1, nc2 = _get_programs()

    x = np.asarray(x, dtype=np.float32)
    wq_h = np.asarray(Wq, dtype=np.float32).astype(F16)
    wk_h = np.asarray(Wk, dtype=np.float32).astype(F16)
    wo32 = np.asarray(Wo, dtype=np.float32)
    woh8, wol8 = _split_e4m3(wo32)
    # [m, p, r, n, f] with planes (hi, lo*32); contract index = n*128+p
    wo8_blk = np.ascontiguousarray(
        np.stack([woh8, wol8], axis=0)                  # [r, (n p), (m f)]
        .reshape(2, NK, 128, NK, 128).transpose(3, 2, 0, 1, 4))
    wo16_blk = np.ascontiguousarray(
        wo32.astype(F16).reshape(NK, 128, NK, 128).transpose(2, 1, 0, 3))
    wvh8, wvl8 = _split_e4m3(np.asarray(Wv, dtype=np.float32))
    # [p, n, r, d] with slots (hi, lo*32)
    wv8 = np.ascontiguousarray(
        np.stack([wvh8, wvl8], axis=0)                  # [r, (n p), d]
        .reshape(2, NK, 128, D).transpose(2, 1, 0, 3))

    # ---- phase 1: per-core row slices ----
    in1 = []
    for c in range(8):
        b, i = divmod(c, 4)
        rows = x[b, i * BLK:(i + 1) * BLK, :]           # [BLK, D]
        xt32 = np.ascontiguousarray(rows.T)             # [D, BLK] fp32
        xh8, xl8 = _split_e4m3(xt32)
        x8 = np.ascontiguousarray(
            np.stack([xh8, xl8], axis=0)                # [r, (n p), s]
            .reshape(2, NK, 128, BLK).transpose(2, 1, 0, 3))
        in1.append({
            "xt": xt32.astype(F16), "x8": x8,
            "wq": wq_h, "wk": wk_h, "wv8": wv8,
        })
    res1 = run_bass_kernel_spmd(nc1, in1, list(range(8))).results

    # ---- host gather of K/V shards into per-batch tensors ----
    kth_full, v_full = [], []
    for b in range(B):
        kth_full.append(np.concatenate(
            [res1[b * 4 + i]["kt"] for i in range(4)], axis=1))    # [D, S]
        v_full.append(np.concatenate(
            [res1[b * 4 + i]["vo"] for i in range(4)], axis=0))    # [NT, 128, D]

    # ---- phase 2 ----
    in2 = []
    for c in range(8):
        b, i = divmod(c, 4)
        qt_c = res1[c]["qt"]                                     # [D, BLK] fp16
        # [n, p, i, f] -> [i, p, n, f]
        qt1_c = np.ascontiguousarray(
            qt_c.reshape(NK, 128, NI, 128).transpose(2, 1, 0, 3))
        in2.append({
            "kth": kth_full[b], "vin": v_full[b],
            "qt1": qt1_c,
            "wo8": wo8_blk, "wo16": wo16_blk,
        })
    res2 = run_bass_kernel_spmd(nc2, in2, list(range(8))).results

    out = np.empty((B, S, D), dtype=np.float32)
    for c in range(8):
        b, i = divmod(c, 4)
        out[b, i * BLK:(i + 1) * BLK, :] = res2[c]["yt"].T
    return out


# revision 117
# speedup vs baseline: 1.0242x; 1.0006x over previous
"""Trainium2 Bass kernel for single-head self-attention (B=2, S=4096, D=1024).

reference:
    q = x @ Wq; k = x @ Wk; v = x @ Wv          # [B,S,D]
    energy = einsum('bid,bjd->bij', q, k) * 8.0  # SCALE = sqrt(64)
    attn = softmax(energy, axis=-1)
    out = einsum('bij,bjd->bid', attn, v) @ Wo

Two SPMD launches over 8 cores (= 2 batches x 4 query-blocks of 1024):
  phase 1: each core computes the Q/K/V projections for its own 1024
           rows only (1/8 of the total work, no redundancy); the host
           gathers K/V shards into full per-batch tensors.
  phase 2: each core runs attention + output projection for its block
           against the full K/V of its batch.

Precision: logits have std ~256 (SCALE multiplies), so softmax is
nearly an argmax -- the x->Q, x->K, Q@K^T path needs ~2^-12 relative
accuracy.  fp16 (e5m10) gives 2^-11.8 rounding at the same 1
cycle/row PE rate as bf16, and Q/K values are ~N(0,1) so the e5
range is ample; those matmuls run as a single fp16 pass with fp32
PSUM accumulation (end-to-end rel err ~8e-3, dominated by the
softmax's sensitivity to logit rounding; bf16 single-pass would be
~5e-2, and the old bf16 hi/lo 3-pass scheme costs 3x the cycles).
The V projection tolerates ~0.5% error, so it runs entirely as fp8
e4m3 DoubleRow matmuls (0.5 cycles/row, two k-tiles contracted per
instruction = 4x bf16 MAC rate in the cost model): hi*hi plus the
two hi*lo cross terms, with the lo planes pre-scaled by 32 to clear
the e4m3 denormal floor and the 1/32 folded back during the
PSUM->SBUF merge (scalar-engine Copy-with-scale + DVE add).

Phase 2 is software-pipelined: each iteration emits softmax(i)'s
Act/DVE chain, then E(i+1)'s matmuls (which keep the PE busy through
the softmax latency), then P^T transposes and P@V for i.  K, V and
the full output projection Wo path are scheduled so V stays resident
in SBUF all phase (the fp16/fp8 shrink of K freed the room), and the
output projection runs as column strips woven into the E sweeps'
PSUM-drain slots, with the last strip packed 4-chunks-per-bank into
the by-then-idle PV banks.

Layout: feature-major ("transposed") activations throughout; the host
pre-transposes x and post-transposes the output. DMA instruction count
is kept low (batched loads/stores) -- each HWDGE descriptor-generation
costs ~0.65us of serialized queue-prep time on its issuing engine, and
all transfers serialize on the DMA-engine pool at ~360 GB/s, so issue
order is chosen to match consumption order (prologue loads split
across the SP and Activation HWDGE queues).
"""

import numpy as np

B, S, D = 2, 4096, 1024
BLK = 1024          # queries per core
SCALE = 8.0         # HEAD_DIM ** 0.5 = sqrt(64)
NK = D // 128       # 8 k-tiles over the feature dim
NT = S // 128       # 32 j-tiles over keys
NI = BLK // 128     # 8 i-tiles over this core's queries
NJB = S // 512      # 8 key blocks of 512
F16 = np.float16
import ml_dtypes
E4M3 = ml_dtypes.float8_e4m3
LO_SCALE = np.float32(32.0)   # pre-scale on e4m3 lo planes (kept out of denormals)


def _split_e4m3(a):
    """fp32 -> (hi, lo) e4m3 planes with lo pre-scaled by LO_SCALE."""
    hi = a.astype(E4M3)
    lo = ((a - hi.astype(np.float32)) * LO_SCALE).astype(E4M3)
    return hi, lo

# phase-2 tuning knobs (swept via TimelineSim); PSUM is 8 banks total:
# EPS + TP + 2 (op0/op1) + YPS <= 8
EPS_BUFS = 3        # PSUM banks for E accumulation
TP_BUFS = 2         # PSUM banks for PE transposes
YPS_BUFS = 1        # PSUM banks for Y (out-projection) accumulation
DUAL_Q = True       # split prologue DMA issue across SP + Act HWDGE queues
FIRST_BAND_512 = False  # 512-col leading K bands instead of 1024
WO_RESIDENT = False  # keep all of Wo in SBUF instead of streaming per strip
QV23 = True         # preload qv2/qv3 on the Act queue
STRIP_MID = True    # Y strips ride as E-sweep fillers (see `fillers`)

_cache = {}


def _build_phase1():
    """Q/K/V projections for this core's 1024 rows."""
    import concourse.mybir as mybir
    from concourse import bacc
    from concourse.tile import TileContext

    FP32 = mybir.dt.float32
    DF16 = mybir.dt.float16
    DF8 = mybir.dt.float8e4
    DR = mybir.MatmulPerfMode.DoubleRow
    ADD = mybir.AluOpType.add
    Copy = mybir.ActivationFunctionType.Copy

    nc = bacc.Bacc("TRN2", target_bir_lowering=False, debug=False, num_devices=8)

    xt = nc.dram_tensor("xt", [D, BLK], DF16, kind="ExternalInput")   # rows.T
    # e4m3 2-level x/Wv for the V projection; lo planes pre-scaled by 32
    x8 = nc.dram_tensor("x8", [128, NK, 2, BLK], DF8, kind="ExternalInput")
    wv8 = nc.dram_tensor("wv8", [128, NK, 2, D], DF8, kind="ExternalInput")
    wq = nc.dram_tensor("wq", [D, D], DF16, kind="ExternalInput")
    wk = nc.dram_tensor("wk", [D, D], DF16, kind="ExternalInput")
    qt = nc.dram_tensor("qt", [D, BLK], DF16, kind="ExternalOutput")
    kt = nc.dram_tensor("kt", [D, BLK], DF16, kind="ExternalOutput")
    vo = nc.dram_tensor("vo", [NI, 128, D], DF16, kind="ExternalOutput")

    with TileContext(nc) as tc:
      with (
          tc.tile_pool(name="xqk", bufs=1) as xqkp,
          tc.tile_pool(name="wkqp", bufs=1) as wkqp,
          tc.tile_pool(name="wvp", bufs=1) as wvp,
          tc.tile_pool(name="vps", bufs=2, space="PSUM") as vps,
          tc.tile_pool(name="vsb", bufs=3) as vsbp,
      ):
        if True:
            # load order = consumption order; first K-weight sliver and x
            # halves first so the K sweep starts as early as possible
            xt_r = xt[:, :].rearrange("(n p) s -> p n s", p=128)
            xs = xqkp.tile([128, NK, BLK], DF16, name="xs", tag="xs")
            w_sb = {}
            wk_sb = wkqp.tile([128, NK, D], DF16, name="kwt", tag="kwt")
            wk_r = wk[:, :].rearrange("(n p) d -> p n d", p=128)
            nc.scalar.dma_start(wk_sb[:, :, 0:128], wk_r[:, :, 0:128])
            nc.sync.dma_start(xs[:, :, 0:128], xt_r[:, :, 0:128])
            nc.sync.dma_start(xs[:, :, 128:256], xt_r[:, :, 128:256])
            nc.scalar.dma_start(wk_sb[:, :, 128:512], wk_r[:, :, 128:512])
            nc.sync.dma_start(xs[:, :, 256:512], xt_r[:, :, 256:512])
            nc.sync.dma_start(xs[:, :, 512:BLK], xt_r[:, :, 512:BLK])
            nc.scalar.dma_start(wk_sb[:, :, 512:D], wk_r[:, :, 512:D])
            w_sb["k"] = wk_sb
            x8_sb = wvp.tile([128, NK, 2, BLK], DF8, name="x8t", tag="x8t")
            nc.scalar.dma_start(x8_sb, x8[:, :, :, :])
            wv8_sb = wvp.tile([128, NK, 2, D], DF8, name="wv8t", tag="wv8t")
            nc.sync.dma_start(wv8_sb, wv8[:, :, :, :])
            wq_sb = wkqp.tile([128, NK, D], DF16, name="qwt", tag="qwt")
            wq_r = wq[:, :].rearrange("(n p) d -> p n d", p=128)
            nc.scalar.dma_start(wq_sb[:, :, 0:512], wq_r[:, :, 0:512])
            nc.sync.dma_start(wq_sb[:, :, 512:D], wq_r[:, :, 512:D])
            w_sb["q"] = wq_sb
        # ------------- Kt, then V, then Qt -------------
        if True:
            def v_section():
                # V = x @ Wv in 2-level e4m3: hi*hi into one PSUM bank, the
                # two cross terms (carrying the x32 lo pre-scale) into a
                # second, merged by one (cross/32 + main) DVE op.  All four
                # products run as fp8 DoubleRow matmuls (0.5 cycles/row,
                # k-tile pairs), 0.75x the fp16 cost.
                NH = NK // 2
                for j in range(NI):
                    vt = vsbp.tile([128, D], DF16, name=f"vt{j}", tag="vt")
                    jsl = slice(j * 128, (j + 1) * 128)
                    for db in range(2):
                        dsl = slice(db * 512, (db + 1) * 512)
                        psm = vps.tile([128, 512], FP32, name=f"vpm{j}_{db}", tag="vpm")
                        psc = vps.tile([128, 512], FP32, name=f"vpc{j}_{db}", tag="vpc")
                        for kp in range(NH):
                            k2 = slice(2 * kp, 2 * kp + 2)
                            nc.tensor.matmul(
                                psm, lhsT=x8_sb[:, k2, 0, jsl],
                                rhs=wv8_sb[:, k2, 0, dsl], perf_mode=DR,
                                start=(kp == 0), stop=(kp == NH - 1))
                        for kp in range(NH):
                            k2 = slice(2 * kp, 2 * kp + 2)
                            nc.tensor.matmul(
                                psc, lhsT=x8_sb[:, k2, 0, jsl],
                                rhs=wv8_sb[:, k2, 1, dsl], perf_mode=DR,
                                start=(kp == 0), stop=False)
                            nc.tensor.matmul(
                                psc, lhsT=x8_sb[:, k2, 1, jsl],
                                rhs=wv8_sb[:, k2, 0, dsl], perf_mode=DR,
                                start=False, stop=(kp == NH - 1))
                        # cross/32 then add, both on DVE (no cross-engine
                        # handshake; walrus rejects dual-PSUM fused forms)
                        vc = vsbp.tile([128, 512], DF16, name=f"vc{j}_{db}", tag="vc")
                        nc.vector.tensor_scalar_mul(vc, psc, float(1.0 / LO_SCALE))
                        nc.vector.tensor_tensor(vt[:, dsl], psm, vc, op=ADD)
                    nc.sync.dma_start(vo[j], vt)

            # K's leading column-chunks are narrow so its first PSUM group
            # only needs 128 columns of x to have landed
            CHUNKS = {"k": ((0, 128), (128, 256), (256, 384), (384, 512), (512, 1024)),
                      "q": ((0, 512), (512, 1024))}
            for (outd, pfx) in ((kt, "k"), (qt, "q")):
                with (
                    tc.tile_pool(name=f"{pfx}ps", bufs=4, space="PSUM") as psp,
                    tc.tile_pool(name=f"{pfx}st", bufs=1) as stp,
                ):
                    w_t = w_sb[pfx]
                    sh_t = []
                    for m in range(NK):
                        sh_t.append(stp.tile([128, BLK], DF16, name=f"{pfx}sh{m}", tag=f"sh{m}"))
                    chunks = CHUNKS[pfx]
                    lc = len(chunks) - 1
                    if pfx == "k":
                        # ordered by operand arrival: narrow x chunks against
                        # the first weight columns first, high-m columns once
                        # the rest of the weight lands, then the wide chunk
                        order = ([(ci, m) for ci in range(4) for m in range(4)]
                                 + [(ci, m) for ci in range(4) for m in range(4, NK)]
                                 + [(lc, m) for m in range(NK)])
                    else:
                        order = [(ci, m) for ci in range(len(chunks)) for m in range(NK)]
                    for (ci, m) in order:
                        c0, c1 = chunks[ci]
                        nsl = slice(c0, c1)
                        msl = slice(m * 128, (m + 1) * 128)
                        ps = psp.tile([128, 512], FP32, name=f"{pfx}ps{ci}_{m}", tag="ps")
                        for k in range(NK):
                            nc.tensor.matmul(ps[:, 0:c1 - c0], lhsT=w_t[:, k, msl],
                                             rhs=xs[:, k, nsl],
                                             start=(k == 0), stop=(k == NK - 1))
                        nc.vector.tensor_copy(sh_t[m][:, nsl], ps[:, 0:c1 - c0])
                        if ci == lc:
                            # store as soon as the last chunk of m is done
                            nc.sync.dma_start(outd[msl, :], sh_t[m])
                if pfx == "k":
                    v_section()
    nc.compile()
    return nc


def _build_phase2():
    """Attention + output projection for this core's 1024 queries."""
    import concourse.mybir as mybir
    from concourse import bacc
    from concourse.tile import TileContext
    from concourse.masks import make_identity

    FP32 = mybir.dt.float32
    DF16 = mybir.dt.float16
    DF8 = mybir.dt.float8e4
    DR = mybir.MatmulPerfMode.DoubleRow
    ADD = mybir.AluOpType.add
    SUB = mybir.AluOpType.subtract
    Exp = mybir.ActivationFunctionType.Exp
    Copy = mybir.ActivationFunctionType.Copy
    AX = mybir.AxisListType.X

    nc = bacc.Bacc("TRN2", target_bir_lowering=False, debug=False, num_devices=8)

    kth = nc.dram_tensor("kth", [D, S], DF16, kind="ExternalInput")
    # per-i-tile partition-major Q: [i, p, n, f] = qt[n*128+p, i*128+f]
    qt1 = nc.dram_tensor("qt1", [NI, 128, NK, 128], DF16, kind="ExternalInput")
    vin = nc.dram_tensor("vin", [NT, 128, D], DF16, kind="ExternalInput")
    # 2-plane e4m3 Wo, lo plane pre-scaled by 32:
    # [m, p, r, n, f] = plane_r[n*128+p, m*128+f]
    wo8 = nc.dram_tensor("wo8", [NK, 128, 2, NK, 128], DF8, kind="ExternalInput")
    # fp16 Wo for the two exposed final strips
    wo16 = nc.dram_tensor("wo16", [NK, 128, NK, 128], DF16, kind="ExternalInput")
    yt = nc.dram_tensor("yt", [D, BLK], DF16, kind="ExternalOutput")

    from contextlib import ExitStack
    with TileContext(nc) as tc:
        with ExitStack() as stack:
            constp = stack.enter_context(tc.tile_pool(name="const", bufs=1))
            ident = constp.tile([128, 128], DF16)
            make_identity(nc, ident)

            ktp = stack.enter_context(tc.tile_pool(name="ktp", bufs=1))
            qtp = stack.enter_context(tc.tile_pool(name="qtp", bufs=3))
            if True:
                # first two i-tiles' Q loads before the big K transfers
                qv0 = qtp.tile([128, NK, 128], DF16, name="qv0", tag="qv")
                nc.sync.dma_start(qv0, qt1[0])
                qv1 = qtp.tile([128, NK, 128], DF16, name="qv1", tag="qv")
                nc.sync.dma_start(qv1, qt1[1])
                # column-banded K loads, issued on BOTH HWDGE queues (SP +
                # Activation) to halve the ~0.65us/DMA descriptor-gen
                # serialization in the prologue; V (resident all phase) and
                # Wo (resident) follow
                kth_sb = []
                for m in range(NK):
                    kth_sb.append(ktp.tile([128, S], DF16, name=f"kth{m}", tag=f"kth{m}"))
                vrp = stack.enter_context(tc.tile_pool(name="vrp", bufs=1))
                vres = vrp.tile([128, NT, D], DF16, name="vres", tag="vres")
                vin_r = vin[:, :, :].rearrange("t p d -> p t d")
                if FIRST_BAND_512:
                    bands = ((0, 512), (512, 1024), (1024, 2048),
                             (2048, 3072), (3072, 4096))
                else:
                    bands = ((0, 1024), (1024, 2048),
                             (2048, 3072), (3072, 4096))
                for bi, (c0, c1) in enumerate(bands):
                    csl = slice(c0, c1)
                    for m in range(NK):
                        eng = nc.scalar if (DUAL_Q and bi < 2 and m >= NK // 2) else nc.sync
                        eng.dma_start(kth_sb[m][:, csl], kth[m * 128:(m + 1) * 128, csl])
                for cb in range(4):
                    tsl = slice(cb * (NT // 4), (cb + 1) * (NT // 4))
                    nc.sync.dma_start(vres[:, tsl], vin_r[:, tsl])
                # everything below is needed only from the middle of the
                # phase on -- keep it BEHIND the V quarters in the DMA FIFO
                qv23 = []
                if QV23:
                    for i in (2, 3):
                        qvn = qtp.tile([128, NK, 128], DF16, name=f"qv{i}", tag="qv")
                        (nc.scalar if DUAL_Q else nc.sync).dma_start(qvn, qt1[i])
                        qv23.append(qvn)
                wop = stack.enter_context(tc.tile_pool(name="wop", bufs=2))

                otp = stack.enter_context(tc.tile_pool(name="otp", bufs=1))
                if True:
                    # O^T in 2-plane e4m3 (lo plane pre-scaled by 32) so the
                    # output projection runs as fp8 DoubleRow matmuls; the
                    # last i-tile's columns stay fp16 (see o_finish)
                    ot8h = otp.tile([128, NK, 768], DF8, name="ot8h", tag="ot8h")
                    ot8l = otp.tile([128, NK, 768], DF8, name="ot8l", tag="ot8l")
                    # fp16 O^T for the last two i-tiles' columns (896:1024 and
                    # 768:896) -- they only feed the two exposed fp16 strips
                    ot16t = otp.tile([128, NK, 256], DF16, name="ot16t", tag="ot16t")
                    epsp = stack.enter_context(tc.tile_pool(name="eps", bufs=EPS_BUFS, space="PSUM"))
                    tpsp = stack.enter_context(tc.tile_pool(name="tps", bufs=TP_BUFS, space="PSUM"))
                    opsp = stack.enter_context(tc.tile_pool(name="ops", bufs=1, space="PSUM"))
                    smp = stack.enter_context(tc.tile_pool(name="smp", bufs=2))
                    esp = stack.enter_context(tc.tile_pool(name="esp", bufs=2))
                    pp = stack.enter_context(tc.tile_pool(name="pp", bufs=1))
                    ptp = stack.enter_context(tc.tile_pool(name="ptp", bufs=1))
                    obp = stack.enter_context(tc.tile_pool(name="obp", bufs=1))
                    ypsp = stack.enter_context(tc.tile_pool(name="yps", bufs=YPS_BUFS, space="PSUM"))
                    ystp = stack.enter_context(tc.tile_pool(name="yst", bufs=1))
                    if True:
                        qvt = {0: qv0, 1: qv1}
                        for j, t in enumerate(qv23):
                            qvt[2 + j] = t

                        def get_qv(i):
                            if i not in qvt:
                                qvt[i] = qtp.tile([128, NK, 128], DF16, name=f"qv{i}", tag="qv")
                                nc.sync.dma_start(qvt[i], qt1[i])
                            return qvt[i]

                        def e_sweep(i, filler=()):
                            """E = Q_i @ K^T for one i-tile (raw logits + per-block
                            maxes).  One `filler` callable runs after each j-block's
                            matmuls -- deferred PE work (Y chunks) slots in while
                            the block's PSUM drains, hiding its bank turnaround."""
                            qv = get_qv(i)
                            if i + 1 < NI:
                                get_qv(i + 1)  # prefetch next Q tile
                            mx8 = smp.tile([128, NJB], FP32, name=f"mx8_{i}", tag="mx8")
                            e_sb = esp.tile([128, S], FP32, name=f"e{i}", tag="e")
                            for jb in range(NJB):
                                sl = slice(jb * 512, (jb + 1) * 512)
                                ps = epsp.tile([128, 512], FP32, name=f"eps{i}_{jb}", tag="eps")
                                for k in range(NK):
                                    nc.tensor.matmul(ps, lhsT=qv[:, k, :],
                                                     rhs=kth_sb[k][:, sl],
                                                     start=(k == 0), stop=(k == NK - 1))
                                nc.vector.tensor_copy(e_sb[:, sl], ps)
                                nc.vector.reduce_max(mx8[:, jb:jb + 1], ps, axis=AX)
                                if jb < len(filler):
                                    filler[jb]()
                            return {"mx8": mx8, "e": e_sb}

                        def sm_exp(i, st):
                            """max-reduce + exp for i-tile (Act/DVE only, no PE)."""
                            mrow = smp.tile([128, 1], FP32, name=f"mrow{i}", tag="mrow")
                            nc.vector.reduce_max(mrow, st["mx8"], axis=AX)
                            negm = smp.tile([128, 1], FP32, name=f"negm{i}", tag="negm")
                            nc.vector.tensor_scalar_mul(negm, mrow, -SCALE)
                            p_sb = pp.tile([128, S], DF16, name=f"p{i}", tag="p")
                            lp8 = smp.tile([128, NJB], FP32, name=f"lp8_{i}", tag="lp8")
                            for jb in range(NJB):
                                sl = slice(jb * 512, (jb + 1) * 512)
                                nc.scalar.activation(
                                    p_sb[:, sl], st["e"][:, sl], Exp,
                                    bias=negm, scale=SCALE,
                                    accum_out=lp8[:, jb:jb + 1],
                                )
                            st["p"], st["lp8"] = p_sb, lp8

                        def sm_sum(i, st):
                            lrow = smp.tile([128, 1], FP32, name=f"lrow{i}", tag="lrow")
                            nc.vector.reduce_sum(lrow, st["lp8"], axis=AX)
                            linv = smp.tile([128, 1], FP32, name=f"linv{i}", tag="linv")
                            nc.vector.reciprocal(linv, lrow)
                            st["linv"] = linv

                        def sm_tp(i, st):
                            """P^T transposes for i-tile, 4 per PSUM tile so one
                            DVE copy moves 4 transposed blocks.  (An XBAR
                            dma_start_transpose was tried here and lost: its
                            ~7us DMA occupancy starves the prologue K/V loads
                            and the Y-filler Wo streams.)"""
                            p_sb = st["p"]
                            pt_sb = ptp.tile([128, NT, 128], DF16, name=f"pt{i}", tag="pt")
                            for g in range(NT // 4):
                                tp = tpsp.tile([128, 4, 128], DF16, name=f"tp{i}_{g}", tag="tp")
                                for w in range(4):
                                    t = g * 4 + w
                                    nc.tensor.transpose(
                                        tp[:, w, :], p_sb[:, t * 128:(t + 1) * 128], ident)
                                nc.vector.tensor_copy(pt_sb[:, g * 4:(g + 1) * 4], tp)
                            st["pt"] = pt_sb

                        NH = NK // 2

                        def y_mms(ps_main, ps_cross, wom, nsl, first, last):
                            # main (hi*hi) and the two cross terms (carrying the
                            # x32 lo pre-scale) as fp8 DoubleRow k-pair matmuls
                            for kp in range(NH):
                                k2 = slice(2 * kp, 2 * kp + 2)
                                nc.tensor.matmul(
                                    ps_main, lhsT=wom[:, 0, k2, :],
                                    rhs=ot8h[:, k2, nsl], perf_mode=DR,
                                    start=first and kp == 0, stop=last and kp == NH - 1)
                            for kp in range(NH):
                                k2 = slice(2 * kp, 2 * kp + 2)
                                nc.tensor.matmul(
                                    ps_cross, lhsT=wom[:, 1, k2, :],
                                    rhs=ot8h[:, k2, nsl], perf_mode=DR,
                                    start=first and kp == 0, stop=False)
                                nc.tensor.matmul(
                                    ps_cross, lhsT=wom[:, 0, k2, :],
                                    rhs=ot8l[:, k2, nsl], perf_mode=DR,
                                    start=False, stop=last and kp == NH - 1)

                        def y_chunk(m, off, w):
                            # Y[m-rows, off:off+w] = sum_k Wo[k,m]^T @ O^T[k, cols]
                            # main group in the low half of the bank, cross
                            # group in the high half; merged by Act(1/32) + add
                            wom = wop.tile([128, 2, NK, 128], DF8, name=f"wo{off}_{m}", tag="wom")
                            nc.sync.dma_start(wom, wo8[m])
                            nsl = slice(off, off + w)
                            ps = ypsp.tile([128, 512], FP32, name=f"yps{m}_{off}", tag="yps")
                            y_mms(ps[:, 0:w], ps[:, 256:256 + w], wom, nsl, True, True)
                            yc = ystp.tile([128, 256], DF16, name=f"yc{m}_{off}", tag="yc")
                            nc.vector.tensor_scalar_mul(yc[:, 0:w], ps[:, 256:256 + w],
                                                        float(1.0 / LO_SCALE))
                            ys = ystp.tile([128, 256], DF16, name=f"ys{m}_{off}", tag="ys")
                            nc.vector.tensor_tensor(ys[:, 0:w], ps[:, 0:w], yc[:, 0:w], op=ADD)
                            nc.sync.dma_start(yt[m * 128:(m + 1) * 128, nsl], ys[:, 0:w])

                        def y_chunks(off, w):
                            return [(lambda m=m: y_chunk(m, off, w)) for m in range(NK)]

                        def y_cols_packed(off, w, tags=("yps",)):
                            # 2 m-chunks (main+cross each) per PSUM bank; a
                            # ring of banks hides the drain turnarounds of an
                            # exposed strip (banks not named "yps" borrow the
                            # PV op0/op1 banks, which are free between
                            # o_finish(i-1) and PV(i)).
                            assert w <= 128
                            nsl = slice(off, off + w)
                            for mg in range(NK // 2):
                                tag = tags[mg % len(tags)]
                                pool = ypsp if tag == "yps" else opsp
                                ps = pool.tile([128, 4, 128], FP32, name=f"yp4_{mg}_{off}",
                                               tag=tag)
                                for w2 in range(2):
                                    m = mg * 2 + w2
                                    wom = wop.tile([128, 2, NK, 128], DF8,
                                                   name=f"wo{off}_{m}", tag="wom")
                                    nc.sync.dma_start(wom, wo8[m])
                                    y_mms(ps[:, 2 * w2, 0:w], ps[:, 2 * w2 + 1, 0:w],
                                          wom, nsl, True, True)
                                yc = ystp.tile([128, 2, 128], DF16, name=f"yc4_{mg}_{off}", tag="yc")
                                nc.vector.tensor_scalar_mul(yc[:, :, 0:w], ps[:, 1::2, 0:w],
                                                            float(1.0 / LO_SCALE))
                                ys = ystp.tile([128, 2, 128], DF16, name=f"ys4_{mg}_{off}", tag="ys4")
                                nc.vector.tensor_tensor(ys[:, :, 0:w], ps[:, 0::2, 0:w],
                                                        yc[:, :, 0:w], op=ADD)
                                # one DMA for the whole 2-m row block
                                nc.sync.dma_start(
                                    yt[mg * 256:(mg + 1) * 256, nsl]
                                    .rearrange("(w p) f -> p w f", p=128),
                                    ys[:, :, 0:w])

                        def y_cols_f16(off, tags):
                            # exposed final strips in fp16: longer per-chunk
                            # compute hides the bank drains; 4 chunks per bank,
                            # one copy + one batched store per 4-m block.  The
                            # fp16 Wo tiles borrow the idle qv pool buffers.
                            nsl = slice(off, off + 128)
                            csl = slice(off - 768, off - 768 + 128)
                            for mg in range(NK // 4):
                                tag = tags[mg % len(tags)]
                                pool = ypsp if tag == "yps" else opsp
                                ps = pool.tile([128, 4, 128], FP32, name=f"yf{mg}_{off}", tag=tag)
                                for w4 in range(4):
                                    m = mg * 4 + w4
                                    wom = qtp.tile([128, NK, 128], DF16,
                                                   name=f"wo16_{off}_{m}", tag="qv")
                                    nc.sync.dma_start(wom, wo16[m])
                                    for k in range(NK):
                                        nc.tensor.matmul(
                                            ps[:, w4, :], lhsT=wom[:, k, :],
                                            rhs=ot16t[:, k, csl],
                                            start=(k == 0), stop=(k == NK - 1),
                                        )
                                ys = ystp.tile([128, 4, 128], DF16, name=f"yf16_{mg}_{off}", tag="ys4")
                                nc.vector.tensor_copy(ys, ps)
                                nc.sync.dma_start(
                                    yt[mg * 512:(mg + 1) * 512, nsl]
                                    .rearrange("(w p) f -> p w f", p=128),
                                    ys)

                        def o_half(i, g, op, linv, osb):
                            """1/l scale + O transpose for one 512-col half of
                            one i-tile (half g aligns with transpose group g)."""
                            isl = slice(i * 128, (i + 1) * 128)
                            nc.vector.tensor_scalar_mul(osb[:, g * 512:(g + 1) * 512], op, linv)
                            g4 = slice(g * 4, (g + 1) * 4)
                            tp = tpsp.tile([128, 4, 128], DF16, name=f"otp{i}_{g}", tag="tp")
                            for w in range(4):
                                t = g * 4 + w
                                nc.tensor.transpose(
                                    tp[:, w, :], osb[:, t * 128:(t + 1) * 128], ident)
                            if i >= NI - 2:
                                t16 = slice((i - (NI - 2)) * 128, (i - (NI - 2)) * 128 + 128)
                                nc.vector.tensor_copy(ot16t[:, g4, t16], tp)
                            else:
                                nc.vector.tensor_copy(ot8h[:, g4, isl], tp)
                                tmp = obp.tile([128, 4, 128], DF16, name=f"otmp{i}_{g}", tag="otmp")
                                nc.vector.tensor_tensor(tmp, tp, ot8h[:, g4, isl], op=SUB)
                                nc.scalar.activation(ot8l[:, g4, isl], tmp, Copy,
                                                     scale=float(LO_SCALE))

                        def o_finish(i, op0, op1, linv):
                            """1/l scale + O transpose for one i-tile.  i<7
                            feeds the 2-plane e4m3 O^T tensors (its columns are
                            consumed by fp8 Y strips mid-phase); the last
                            i-tile keeps a short fp16 path since its columns
                            only feed the exposed final strip, where the e4m3
                            split chain would sit on the critical tail."""
                            isl = slice(i * 128, (i + 1) * 128)
                            osb = obp.tile([128, D], DF16, name=f"osb{i}", tag="osb")
                            nc.vector.tensor_scalar_mul(osb[:, 0:512], op0, linv)
                            nc.vector.tensor_scalar_mul(osb[:, 512:D], op1, linv)
                            for g in range(NK // 4):
                                g4 = slice(g * 4, (g + 1) * 4)
                                tp = tpsp.tile([128, 4, 128], DF16, name=f"otp{i}_{g}", tag="tp")
                                for w in range(4):
                                    t = g * 4 + w
                                    nc.tensor.transpose(
                                        tp[:, w, :], osb[:, t * 128:(t + 1) * 128], ident)
                                if i >= NI - 2:
                                    t16 = slice((i - (NI - 2)) * 128, (i - (NI - 2)) * 128 + 128)
                                    nc.vector.tensor_copy(ot16t[:, g4, t16], tp)
                                else:
                                    nc.vector.tensor_copy(ot8h[:, g4, isl], tp)
                                    tmp = obp.tile([128, 4, 128], DF16, name=f"otmp{i}_{g}", tag="otmp")
                                    nc.vector.tensor_tensor(tmp, tp, ot8h[:, g4, isl], op=SUB)
                                    nc.scalar.activation(ot8l[:, g4, isl], tmp, Copy,
                                                         scale=float(LO_SCALE))

                        if True:
                            # software pipeline: E(i+1) is emitted between
                            # softmax(i)'s Act/DVE chain and the P^T/PV(i) PE
                            # work, so the PE never idles waiting on softmax
                            # Y strips ride along as E-sweep fillers once their
                            # o_finish dependencies are met: E(i+1) is emitted in
                            # iteration i, so strip cols [c, c+256) (i-tiles
                            # c/128..c/128+1) can fill E(c/128+2)'s sweep
                            fillers = {5: y_chunks(0, 256), 6: y_chunks(256, 256),
                                       7: y_chunks(512, 256)}
                            st = {0: e_sweep(0)}
                            for i in range(NI):
                                s = st.pop(i)
                                sm_exp(i, s)
                                if i + 1 < NI:
                                    st[i + 1] = e_sweep(i + 1, fillers.get(i + 1, ()))
                                else:
                                    # no E to fill the last softmax window --
                                    # use the last even-numbered strip instead
                                    y_cols_f16(768, tags=("yps",))
                                sm_sum(i, s)
                                sm_tp(i, s)
                                pt_sb = s["pt"]
                                if i == NI - 1:
                                    # sequential PV halves so half0's scale/
                                    # transpose/copy chain overlaps half1's
                                    # matmuls -- shortens the exposed tail
                                    osb = obp.tile([128, D], DF16, name=f"osb{i}", tag="osb")
                                    op0 = opsp.tile([128, 512], FP32, name=f"op0_{i}", tag="op0")
                                    for t in range(NT):
                                        nc.tensor.matmul(
                                            op0, lhsT=pt_sb[:, t, :], rhs=vres[:, t, 0:512],
                                            start=(t == 0), stop=(t == NT - 1))
                                    o_half(i, 0, op0, s["linv"], osb)
                                    op1 = opsp.tile([128, 512], FP32, name=f"op1_{i}", tag="op1")
                                    for t in range(NT):
                                        nc.tensor.matmul(
                                            op1, lhsT=pt_sb[:, t, :], rhs=vres[:, t, 512:D],
                                            start=(t == 0), stop=(t == NT - 1))
                                    o_half(i, 1, op1, s["linv"], osb)
                                else:
                                    op0 = opsp.tile([128, 512], FP32, name=f"op0_{i}", tag="op0")
                                    op1 = opsp.tile([128, 512], FP32, name=f"op1_{i}", tag="op1")
                                    for t in range(NT):
                                        nc.tensor.matmul(
                                            op0, lhsT=pt_sb[:, t, :], rhs=vres[:, t, 0:512],
                                            start=(t == 0), stop=(t == NT - 1))
                                        nc.tensor.matmul(
                                            op1, lhsT=pt_sb[:, t, :], rhs=vres[:, t, 512:D],
                                            start=(t == 0), stop=(t == NT - 1))
                                    o_finish(i, op0, op1, s["linv"])
                            y_cols_f16(896, tags=("op0", "op1"))
    nc.compile()
    return nc


def _get_programs():
    if "nc1" not in _cache:
        _cache["nc1"] = _build_phase1()
        _cache["nc2"] = _build_phase2()
    return _cache["nc1"], _cache["nc2"]


def kernel(x, Wq, Wk, Wv, Wo):
    from concourse.bass_utils import run_bass_kernel_spmd

    nc1, nc2 = _get_programs()

    x = np.asarray(x, dtype=np.float32)
    wq_h = np.asarray(Wq, dtype=np.float32).astype(F16)
    wk_h = np.asarray(Wk, dtype=np.float32).astype(F16)
    wo32 = np.asarray(Wo, dtype=np.float32)
    woh8, wol8 = _split_e4m3(wo32)
    # [m, p, r, n, f] with planes (hi, lo*32); contract index = n*128+p
    wo8_blk = np.ascontiguousarray(
        np.stack([woh8, wol8], axis=0)                  # [r, (n p), (m f)]
        .reshape(2, NK, 128, NK, 128).transpose(3, 2, 0, 1, 4))
    wo16_blk = np.ascontiguousarray(
        wo32.astype(F16).reshape(NK, 128, NK, 128).transpose(2, 1, 0, 3))
    wvh8, wvl8 = _split_e4m3(np.asarray(Wv, dtype=np.float32))
    # [p, n, r, d] with slots (hi, lo*32)
    wv8 = np.ascontiguousarray(
        np.stack([wvh8, wvl8], axis=0)                  # [r, (n p), d]
        .reshape(2, NK, 128, D).transpose(2, 1, 0, 3))

    # ---- phase 1: per-core row slices ----
    in1 = []
    for c in range(8):
        b, i = divmod(c, 4)
        rows = x[b, i * BLK:(i + 1) * BLK, :]           # [BLK, D]
        xt32 = np.ascontiguousarray(rows.T)             # [D, BLK] fp32
        xh8, xl8 = _split_e4m3(xt32)
        x8 = np.ascontiguousarray(
            np.stack([xh8, xl8], axis=0)                # [r, (n p), s]
            .reshape(2, NK, 128, BLK).transpose(2, 1, 0, 3))
        in1.append({
            "xt": xt32.astype(F16), "x8": x8,
            "wq": wq_h, "wk": wk_h, "wv8": wv8,
        })
    res1 = run_bass_kernel_spmd(nc1, in1, list(range(8))).results

    # ---- host gather of K/V shards into per-batch tensors ----
    kth_full, v_full = [], []
    for b in range(B):
        kth_full.append(np.concatenate(
            [res1[b * 4 + i]["kt"] for i in range(4)], axis=1))    # [D, S]
        v_full.append(np.concatenate(
            [res1[b * 4 + i]["vo"] for i in range(4)], axis=0))    # [NT, 128, D]

    # ---- phase 2 ----
    in2 = []
    for c in range(8):
        b, i = divmod(c, 4)
        qt_c = res1[c]["qt"]                                     # [D, BLK] fp16
        # [n, p, i, f] -> [i, p, n, f]
        qt1_c = np.ascontiguousarray(
            qt_c.reshape(NK, 128, NI, 128).transpose(2, 1, 0, 3))
        in2.append({
            "kth": kth_full[b], "vin": v_full[b],
            "qt1": qt1_c,
            "wo8": wo8_blk, "wo16": wo16_blk,
        })
    res2 = run_bass_kernel_spmd(nc2, in2, list(range(8))).results

    out = np.empty((B, S, D), dtype=np.float32)
    for c in range(8):
        b, i = divmod(c, 4)
        out[b, i * BLK:(i + 1) * BLK, :] = res2[c]["yt"].T
    return out
